# revision 20
# baseline (speedup 1.0000x reference)
"""Trainium2 Bass kernel for nn_Decoder (LSTM decoder + attention + copy mechanism).

Strategy: pure batch-parallel across the 8 NeuronCores — each core runs the
full T=48-step recurrence and the vocab projection for its 4 batch elements,
with zero cross-core communication (this runtime exposes none).

The recurrence runs in feature-major layout: gates/hidden/cell live as
[feature-chunk(128-part), batch] tiles, the gate weights are the STATIONARY
matmul operand (lhsT, fp8 e3m4 resident in SBUF) and the batch-4 activations
stream as the 4-column moving operand, so a gate matmul costs 4 PE rows
instead of 512. tanh(g) is folded into one full-width sigmoid by pre-doubling
the g-gate weight rows on the host (tanh(x) = 2*sigmoid(2x)-1). The per-step
emission is software-pipelined: the next step's embedding/h0 gate matmuls and
layer-1 h1-part fill the PE while the current step's cell updates and
attention softmax run on Act/DVE/Pool.

All large DMA transfers are single instructions over host-prelaid [128, N]
images (HWDGE fixed cost ~650ns each makes many small DMAs expensive).

Self-contained: builds the Bass program, shards inputs on the host, runs via
run_bass_kernel_spmd on cores 0-7, reassembles the full [T, B, V] output.
"""
import sys

sys.path.insert(0, "/opt/trn_rl_repo")

import numpy as np
import ml_dtypes

import concourse.bass as bass
import concourse.mybir as mybir
import concourse.tile as tile
from concourse.bass_utils import run_bass_kernel_spmd

F32 = mybir.dt.float32
BF16 = mybir.dt.bfloat16
FP8 = mybir.dt.float8e3
I16 = mybir.dt.int16
AF = mybir.ActivationFunctionType
ALU = mybir.AluOpType

nbf16 = ml_dtypes.bfloat16
nfp8 = ml_dtypes.float8_e3m4
WS = 64.0                   # fp8 weight pre-scale (compensated in activations)
DR = mybir.MatmulPerfMode.DoubleRow

V, E, H = 10000, 512, 1024
T, S, B = 48, 48, 32
PAD, COPY_ID, EPS = 0, 1, 1e-7
NCORES = 8
BL = B // NCORES            # batch per core = 4
G4 = 4 * H                  # 4096 gate width
NVC = 20                    # vocab chunks of 512
VCH = 512
KC_E = E // 128             # 4
KC_H = H // 128             # 8
VKC8 = 80                   # padded vocab chunks (8 super-chunks of 10)

# psum column base per gate type (torch order i,f,g,o), laid out i|f|o|2g so
# the three sigmoids and doubled-g all go through one [0:128] sigmoid
_GCOL = {0: 0, 1: 32, 2: 96, 3: 64}


def _gcol(m):
    return _GCOL[m // 8] + (m % 8) * BL


# ---------------------------------------------------------------- wait split
def _split_wide_waits(nc):
    """walrus CTRL codegen accepts at most 1 sync-wait per instruction; move
    excess waits onto preceding NoOps on the same (in-order) engine."""
    for f in nc.m.functions:
        for bb in f.blocks:
            ins_list = list(bb.instructions)
            out = []
            changed = False
            for ins in ins_list:
                si = getattr(ins, "sync_info", None)
                waits = list(si.on_wait) if si is not None else []
                if len(waits) > 1:
                    excess, keep = waits[:-1], waits[-1:]
                    for w in excess:
                        nop = mybir.InstNoOp(
                            name=f"I-{nc.next_id()}",
                            opcode="NoOp",
                            engine=ins.engine,
                            debug=ins.debug,
                            ins=[],
                            outs=[],
                            sync_info=mybir.SyncInfo(on_wait=[w], on_update=[]),
                        )
                        try:
                            nc.register_instruction(nop, overwrite=True)
                        except Exception:
                            pass
                        out.append(nop)
                        changed = True
                    si.on_wait = keep
                    ins.sync_info = si
                out.append(ins)
            if changed:
                try:
                    bb.instructions = out
                except Exception:
                    bb.instructions.clear()
                    bb.instructions.extend(out)


# ---------------------------------------------------------------- program
def build_program(t_steps=T):
    nc = bass.Bass("TRN2")
    dp = nc.declare_dram_parameter

    NR = t_steps * BL
    mtiles = [(r0, min(128, NR - r0)) for r0 in range(0, NR, 128)]
    NMT = len(mtiles)

    # all weight images are host-prelaid as a flat [128, N] SBUF image
    wfh_d = dp("wfh", [128, KC_H * G4], FP8, isOutput=False)  # (Wf@Wc)[:, :H]^T
    wfs_d = dp("wfs", [16, 128, 2048], FP8, isOutput=False)   # (Wf@Wc)[:, H:]^T
    wh08_d = dp("wh08", [128, KC_H * G4], FP8, isOutput=False)   # W_hh0^T
    wi18_d = dp("wi18", [128, KC_H * G4], FP8, isOutput=False)   # W_ih1^T
    wh18_d = dp("wh18", [128, KC_H * G4], FP8, isOutput=False)   # W_hh1^T
    we08_d = dp("we08", [128, KC_E * G4], FP8, isOutput=False)   # W_ih0[:,:E]^T
    wcb_d = dp("wcb", [128, 2 * KC_H * H], FP8, isOutput=False)  # Wc^T
    wpb_d = dp("wpb", [NVC, 128, KC_H * VCH], FP8, isOutput=False)  # Wp^T by vc
    wkT_d = dp("wkT", [KC_H, 128, KC_H * 128], BF16, isOutput=False)  # Wk^T by mt
    emb_d = dp("emb8", [16, 128, 5 * E], FP8, isOutput=False)   # embed^T chunks
    encIA_d = dp("encIA", [128, H], BF16, isOutput=False)  # enc rows (s*4+b), s<32
    encIB_d = dp("encIB", [64, H], BF16, isOutput=False)   # s in 32..47
    encT_d = dp("encT", [128, KC_H * BL * S], BF16, isOutput=False)
    reftok_d = dp("reftok", [128, NR], F32, isOutput=False)
    vidx_d = dp("vidx", [128, VKC8], F32, isOutput=False)       # p + 128*ch
    iota512_d = dp("iota512", [128, VCH], F32, isOutput=False)
    srcsh_d = dp("srcsh", [128, 2 * NVC], F32, isOutput=False)  # rows (s*4+b)
    pen_d = dp("pen", [BL, S * BL], BF16, isOutput=False)       # penalty incl mask
    h0T_d = dp("h0T", [128, KC_H * BL], FP8, isOutput=False)
    h1T_d = dp("h1T", [128, KC_H * BL], FP8, isOutput=False)
    c0T_d = dp("c0T", [128, KC_H * BL], F32, isOutput=False)
    c1T_d = dp("c1T", [128, KC_H * BL], F32, isOutput=False)
    ident4_d = dp("ident4", [4, 4], BF16, isOutput=False)

    y_d = dp("y", [128, NMT, V], BF16, isOutput=True)  # host reorders + casts

    with tile.TileContext(nc) as tc:
        with tc.tile_pool(name="wres", bufs=1) as wpool, \
             tc.tile_pool(name="dram", bufs=1, space="DRAM") as dpool:

            e_dram = dpool.tile([128, NMT, NVC * VCH], BF16, name="e_dram")

            dma = nc.sync.dma_start

            # ---- outer-resident (survive into phase 2)
            combT = wpool.tile([128, KC_H, NR], FP8, name="combT")
            dsbA = wpool.tile([128, NR], BF16, name="dsbA")
            dsbB = wpool.tile([64, NR], BF16, name="dsbB")
            zbuf = wpool.tile([128, 2 * NVC], F32, name="zbuf")
            cwn = wpool.tile([128, 2], F32, name="cwn")
            cw = wpool.tile([128, 2], F32, name="cw")
            spp = wpool.tile([128, 2], F32, name="spp")
            ceps = wpool.tile([128, 2], F32, name="ceps")
            ident4 = wpool.tile([4, 4], BF16, name="ident4")
            srcsh = wpool.tile([128, 2 * NVC], F32, name="srcsh")
            iota512 = wpool.tile([128, VCH], F32, name="iota512")
            dma(out=ident4[:], in_=ident4_d[:])
            dma(out=srcsh[:], in_=srcsh_d[:])
            dma(out=iota512[:], in_=iota512_d[:])

            # ======== phases 0+1 (scoped pool; weights freed before phase 2)
            with tc.tile_pool(name="ph01", bufs=1) as p1:
                wfh = p1.tile([128, KC_H, G4], FP8, name="wfh")
                QA = p1.tile([128, 32 * 128], BF16, name="QA")
                QB = p1.tile([64, 32 * 128], BF16, name="QB")
                wh0 = p1.tile([128, KC_H, G4], FP8, name="wh0")
                wi1 = p1.tile([128, KC_H, G4], FP8, name="wi1")
                wh1 = p1.tile([128, KC_H, G4], FP8, name="wh1")
                we0 = p1.tile([128, KC_E, G4], FP8, name="we0")
                wcb = p1.tile([128, 2 * KC_H, H], FP8, name="wcb")
                XeT = p1.tile([128, KC_E, NR], FP8, name="XeT")
                attKT = p1.tile([128, KC_H, BL * S], FP8, name="attKT")
                encIA = p1.tile([128, H], BF16, name="encIA")
                encIB = p1.tile([64, H], BF16, name="encIB")
                pen = p1.tile([BL, S * BL], BF16, name="pen")
                h0T = p1.tile([128, KC_H, BL], FP8, name="h0T")
                h1T = p1.tile([128, KC_H, BL], FP8, name="h1T")
                c0T = p1.tile([128, KC_H * BL], F32, name="c0T")
                c1T = p1.tile([128, KC_H * BL], F32, name="c1T")
                combT0 = p1.tile([128, KC_H, BL], FP8, name="combT0")
                sumT = p1.tile([128, KC_H, BL], FP8, name="sumT")

                # small state first, then weights in first-use order
                dma(out=h0T[:], in_=h0T_d[:])
                dma(out=h1T[:], in_=h1T_d[:])
                dma(out=c0T[:], in_=c0T_d[:])
                dma(out=c1T[:], in_=c1T_d[:])
                dma(out=pen[:], in_=pen_d[:])
                dma(out=encIA[:], in_=encIA_d[:])
                dma(out=encIB[:], in_=encIB_d[:])
                nc.vector.memset(combT0[:], 0.0)

                # ---- phase 0a: X_embT = embed^T @ onehot(ref_tokens)
                with tc.tile_pool(name="ph0", bufs=1) as p0:
                    with tc.tile_pool(name="ph0a", bufs=1) as p0a, \
                         tc.tile_pool(name="ps0a", bufs=1, space="PSUM") as ps0a:
                        reftok = p0a.tile([128, NR], F32, name="reftok")
                        vidx = p0a.tile([128, VKC8], F32, name="vidx")
                        dma(out=reftok[:], in_=reftok_d[:])
                        dma(out=vidx[:], in_=vidx_d[:])
                        psX = [ps0a.tile([128, NR], F32, name=f"psX{m}",
                                         tag=f"psX{m}", bufs=1)
                               for m in range(KC_E)]
                        for sc in range(16):
                            emb8 = p0a.tile([128, 5 * E], FP8, name="emb8",
                                            tag="emb8", bufs=2)
                            dma(out=emb8[:], in_=emb_d[sc])
                            for j in range(5):
                                ch = sc * 5 + j
                                oref = p0a.tile([128, NR], BF16, name="oref",
                                                tag="oref", bufs=2)
                                nc.vector.tensor_scalar(
                                    out=oref[:], in0=reftok[:],
                                    scalar1=vidx[:, ch:ch + 1],
                                    scalar2=None, op0=ALU.is_equal)
                                for m in range(KC_E):
                                    nc.tensor.matmul(
                                        psX[m][:],
                                        lhsT=emb8[:, j * E + m * 128:
                                                  j * E + (m + 1) * 128],
                                        rhs=oref[:], start=(ch == 0),
                                        stop=(ch == VKC8 - 1))
                        for m in range(KC_E):
                            nc.vector.tensor_scalar(out=XeT[:, m, :],
                                                    in0=psX[m][:],
                                                    scalar1=1.0 / WS,
                                                    scalar2=None, op0=ALU.mult)

                    # gate weights (one DMA each, first-use order)
                    dma(out=wh0[:], in_=wh08_d[:])
                    dma(out=we0[:], in_=we08_d[:])
                    dma(out=wh1[:], in_=wh18_d[:])
                    dma(out=wi1[:], in_=wi18_d[:])

                    encTs = p0.tile([128, KC_H * BL * S], BF16, name="encTs")
                    dma(out=encTs[:], in_=encT_d[:])

                    # ---- phase 0b: Q^T = WS * enc @ Wfc_s^T  (two jc passes)
                    with tc.tile_pool(name="ps0q", bufs=1, space="PSUM") as ps0q:
                        for ph in range(2):
                            psq = [ps0q.tile([128, VCH], F32, name=f"psq{i}",
                                             tag=f"psq{i}", bufs=1)
                                   for i in range(8)]
                            qtiles = [(0, 128), (128, 64)]
                            for k in range(KC_H):
                                wfsk = p0.tile([128, 2048], FP8, name="wfsk",
                                               tag="wfsk", bufs=2)
                                dma(out=wfsk[:], in_=wfs_d[ph * KC_H + k])
                                for mt2, (r0, mm) in enumerate(qtiles):
                                    for jc in range(4):
                                        nc.tensor.matmul(
                                            psq[mt2 * 4 + jc][:mm, :],
                                            lhsT=encTs[:, k * BL * S + r0:
                                                       k * BL * S + r0 + mm],
                                            rhs=wfsk[:, jc * VCH:(jc + 1) * VCH],
                                            start=(k == 0), stop=(k == KC_H - 1))
                            for mt2, (r0, mm) in enumerate(qtiles):
                                qdst = QA if mt2 == 0 else QB
                                for jc in range(4):
                                    nc.vector.tensor_copy(
                                        out=qdst[:mm, (ph * 4 + jc) * VCH:
                                                 (ph * 4 + jc + 1) * VCH],
                                        in_=psq[mt2 * 4 + jc][:mm, :])

                    dma(out=wfh[:], in_=wfh_d[:])

                    # ---- phase 0c: att_keyT = Wk @ enc^T
                    with tc.tile_pool(name="ps0c", bufs=1, space="PSUM") as ps0c:
                        for mt in range(KC_H):
                            wkmt = p0.tile([128, KC_H * 128], BF16, name="wkmt",
                                           tag="wkmt", bufs=1)
                            dma(out=wkmt[:], in_=wkT_d[mt])
                            psa = ps0c.tile([128, BL * S], F32, name="psa",
                                            tag="psa", bufs=2)
                            for k in range(KC_H):
                                nc.tensor.matmul(
                                    psa[:], lhsT=wkmt[:, k * 128:(k + 1) * 128],
                                    rhs=encTs[:, k * BL * S:(k + 1) * BL * S],
                                    start=(k == 0), stop=(k == KC_H - 1))
                            nc.vector.tensor_copy(out=attKT[:, mt, :], in_=psa[:])

                    dma(out=wcb[:], in_=wcb_d[:])

                # ======== phase 1: software-pipelined recurrence
                SIG, TANH = AF.Sigmoid, AF.Tanh
                with tc.tile_pool(name="gps", bufs=3, space="PSUM") as gps, \
                     tc.tile_pool(name="sps", bufs=1, space="PSUM") as sps:

                    g0t = {}
                    g1t = {}

                    def getg(d, t):
                        if t not in d:
                            d[t] = gps.tile([128, 128], F32, name="g", tag="g",
                                            bufs=4)
                        return d[t]

                    def drmm(g, w, rhs3, kp, m, start, stop):
                        c = _gcol(m)
                        nc.tensor.matmul(
                            g[:, c:c + BL],
                            lhsT=w[:, 2 * kp:2 * kp + 1, m * 128:(m + 1) * 128],
                            rhs=rhs3[:, 0:1, :], start=start, stop=False)
                        nc.tensor.matmul(
                            g[:, c:c + BL],
                            lhsT=w[:, 2 * kp + 1:2 * kp + 2, m * 128:(m + 1) * 128],
                            rhs=rhs3[:, 1:2, :], start=False, stop=stop)

                    def emit_A(t, part):
                        g0 = getg(g0t, t)
                        if part == "xe":
                            w, kk = we0, KC_E // 2
                            rf = lambda kp: XeT[:, 2 * kp:2 * kp + 2,
                                                t * BL:(t + 1) * BL]
                        elif part == "h0":
                            w, kk = wh0, KC_H // 2
                            rf = lambda kp: h0T[:, 2 * kp:2 * kp + 2, :]
                        else:  # "fh": Wfc_h @ h1^{t-1}
                            w, kk = wfh, KC_H // 2
                            rf = lambda kp: h1T[:, 2 * kp:2 * kp + 2, :]
                        first = part == "xe"
                        last = part == "h0" and t == 0
                        for kp in range(kk):
                            rhs = rf(kp)
                            for m in range(32):
                                drmm(g0, w, rhs, kp, m,
                                     first and kp == 0 and m == 0,
                                     last and kp == kk - 1 and m == 31)

                    def emit_Q(t, tdist):  # gates0 += Q^T @ dist^{tdist}
                        g0 = getg(g0t, t)
                        for m in range(32):
                            c = _gcol(m)
                            nc.tensor.matmul(
                                g0[:, c:c + BL], lhsT=QA[:, m * 128:(m + 1) * 128],
                                rhs=dsbA[:, tdist * BL:(tdist + 1) * BL],
                                start=False, stop=False)
                            nc.tensor.matmul(
                                g0[:, c:c + BL], lhsT=QB[:, m * 128:(m + 1) * 128],
                                rhs=dsbB[:, tdist * BL:(tdist + 1) * BL],
                                start=False, stop=(m == 31))

                    def emit_B(t):  # L1 gates, wh1 @ h1^{t-1}
                        g1 = getg(g1t, t)
                        for kp in range(KC_H // 2):
                            rhs = h1T[:, 2 * kp:2 * kp + 2, :]
                            for m in range(32):
                                drmm(g1, wh1, rhs, kp, m, kp == 0 and m == 0,
                                     False)

                    def emit_C(t):  # L1 gates, wi1 @ h0^t
                        g1 = getg(g1t, t)
                        for kp in range(KC_H // 2):
                            rhs = h0T[:, 2 * kp:2 * kp + 2, :]
                            for m in range(32):
                                drmm(g1, wi1, rhs, kp, m, False,
                                     kp == KC_H // 2 - 1 and m == 31)

                    def emit_cell(t, layer):
                        g = (g0t if layer == 0 else g1t).pop(t)
                        cT = c0T if layer == 0 else c1T
                        hT = h0T if layer == 0 else h1T
                        gs = p1.tile([128, 128], F32, name="gs", tag="gs", bufs=2)
                        nc.scalar.activation(out=gs[:], in_=g[:], func=SIG,
                                             scale=1.0 / WS)
                        gg = p1.tile([128, 32], F32, name="gg", tag="gg", bufs=2)
                        nc.vector.tensor_scalar(out=gg[:], in0=gs[:, 96:128],
                                                scalar1=2.0, scalar2=-1.0,
                                                op0=ALU.mult, op1=ALU.add)
                        u = p1.tile([128, 32], F32, name="u", tag="u", bufs=2)
                        nc.vector.tensor_tensor(out=u[:], in0=gs[:, 0:32],
                                                in1=gg[:], op=ALU.mult)
                        nc.vector.tensor_tensor(out=cT[:], in0=gs[:, 32:64],
                                                in1=cT[:], op=ALU.mult)
                        nc.vector.tensor_tensor(out=cT[:], in0=cT[:], in1=u[:],
                                                op=ALU.add)
                        th = p1.tile([128, 32], F32, name="th", tag="th", bufs=2)
                        nc.scalar.activation(out=th[:], in_=cT[:], func=TANH)
                        nc.vector.tensor_tensor(out=hT[:, :, :], in0=gs[:, 64:96],
                                                in1=th[:], op=ALU.mult)

                    def emit_att(t):
                        pss = sps.tile([BL, BL * S], F32, name="pss", tag="pss",
                                       bufs=1)
                        for k in range(KC_H):
                            nc.tensor.matmul(
                                pss[:], lhsT=h1T[:, k:k + 1, :],
                                rhs=attKT[:, k:k + 1, :],
                                start=(k == 0), stop=False)
                        nc.tensor.matmul(pss[:], lhsT=ident4[:], rhs=pen[:],
                                         start=False, stop=True)
                        ssum = p1.tile([BL, 1], F32, name="ssum", tag="ssum",
                                       bufs=2)
                        dstc = p1.tile([BL, S * BL], F32, name="dstc", tag="dstc",
                                       bufs=2)
                        nc.scalar.activation(out=dstc[:], in_=pss[:], func=AF.Exp,
                                             accum_out=ssum[:])
                        rs = p1.tile([BL, 1], F32, name="rs", tag="rs", bufs=2)
                        nc.vector.reciprocal(out=rs[:], in_=ssum[:])
                        dstb = p1.tile([BL, S * BL], BF16, name="dstb", tag="dstb",
                                       bufs=2)
                        nc.vector.tensor_scalar(out=dstb[:], in0=dstc[:],
                                                scalar1=rs[:], scalar2=None,
                                                op0=ALU.mult)
                        return dstb

                    def emit_sumcomb(t, dstb):
                        psD = sps.tile([128, 2 * BL], BF16, name="psD",
                                       tag="psT", bufs=1)
                        psDA = psD[:, 0:BL]
                        nc.tensor.transpose(psDA, dstb[:, 0:128], ident4[:])
                        nc.vector.tensor_copy(out=dsbA[:, t * BL:(t + 1) * BL],
                                              in_=psDA)
                        psDB = psD[0:64, BL:2 * BL]
                        nc.tensor.transpose(psDB, dstb[:, 128:192], ident4[:])
                        nc.vector.tensor_copy(out=dsbB[:, t * BL:(t + 1) * BL],
                                              in_=psDB)
                        pssu = sps.tile([128, KC_H * BL], F32, name="pssu",
                                        tag="pssu", bufs=1)
                        for j in range(KC_H):
                            nc.tensor.matmul(
                                pssu[:, j * BL:(j + 1) * BL],
                                lhsT=encIA[:, j * 128:(j + 1) * 128],
                                rhs=dsbA[:, t * BL:(t + 1) * BL],
                                start=(j == 0), stop=False)
                            nc.tensor.matmul(
                                pssu[:, j * BL:(j + 1) * BL],
                                lhsT=encIB[:, j * 128:(j + 1) * 128],
                                rhs=dsbB[:, t * BL:(t + 1) * BL],
                                start=False, stop=(j == KC_H - 1))
                        nc.vector.tensor_copy(out=sumT[:, :, :], in_=pssu[:])
                        psc = sps.tile([128, KC_H * BL], F32, name="psc",
                                       tag="psc", bufs=1)
                        for k in range(KC_H):
                            rhs = h1T[:, k:k + 1, :]
                            for mcc in range(KC_H):
                                nc.tensor.matmul(
                                    psc[:, mcc * BL:(mcc + 1) * BL],
                                    lhsT=wcb[:, k:k + 1,
                                             mcc * 128:(mcc + 1) * 128],
                                    rhs=rhs, start=(k == 0 and mcc == 0),
                                    stop=False)
                        for k in range(KC_H):
                            rhs = sumT[:, k:k + 1, :]
                            for mcc in range(KC_H):
                                nc.tensor.matmul(
                                    psc[:, mcc * BL:(mcc + 1) * BL],
                                    lhsT=wcb[:, KC_H + k:KC_H + k + 1,
                                             mcc * 128:(mcc + 1) * 128],
                                    rhs=rhs, start=False,
                                    stop=(k == KC_H - 1 and mcc == KC_H - 1))
                        nc.vector.tensor_scalar(
                            out=combT[:, :, t * BL:(t + 1) * BL], in0=psc[:],
                            scalar1=1.0 / WS, scalar2=None, op0=ALU.mult)

                    for t in range(t_steps):
                        if t == 0:
                            emit_A(0, "xe")
                            emit_A(0, "h0")
                            emit_B(0)
                        emit_cell(t, 0)
                        emit_C(t)
                        if t + 1 < t_steps:
                            emit_A(t + 1, "xe")
                            emit_A(t + 1, "h0")
                        emit_cell(t, 1)
                        if t + 1 < t_steps:
                            emit_A(t + 1, "fh")
                        dstb = emit_att(t)
                        if t + 1 < t_steps:
                            emit_B(t + 1)
                        emit_sumcomb(t, dstb)
                        if t + 1 < t_steps:
                            emit_Q(t + 1, t)

            # ======== phase 2 (own pools; vc outer so Wp streams once)
            with tc.tile_pool(name="ph2", bufs=1) as p2, \
                 tc.tile_pool(name="ps2", bufs=2, space="PSUM") as ps2:
                # one-hot tiles, SBUF-resident fp8 (generated here on DVE)
                ohA = p2.tile([128, NVC * VCH], BF16, name="ohA")
                ohB = p2.tile([64, NVC * VCH], BF16, name="ohB")
                for tl, nrow, oh in ((0, 128, ohA), (1, 64, ohB)):
                    for ch in range(NVC):
                        nc.vector.tensor_scalar(
                            out=oh[:nrow, ch * VCH:(ch + 1) * VCH],
                            in0=iota512[:nrow, :],
                            scalar1=srcsh[:nrow, tl * NVC + ch:tl * NVC + ch + 1],
                            scalar2=None, op0=ALU.is_equal)

                for vc in range(NVC):
                    vlim = min(VCH, V - vc * VCH)
                    wpc = p2.tile([128, KC_H, VCH], FP8, name="wpc", tag="wpc",
                                  bufs=2)
                    dma(out=wpc[:], in_=wpb_d[vc])
                    esb = p2.tile([128, NMT, VCH], BF16, name="esb", tag="esb",
                                  bufs=2)
                    for mt, (r0, mm) in enumerate(mtiles):
                        if mm < 128:
                            nc.vector.memset(esb[:, mt, :], 0.0)
                        psp = ps2.tile([128, VCH], F32, name="psp", tag="psg",
                                       bufs=2)
                        for k in range(KC_H):
                            nc.tensor.matmul(
                                psp[:mm, :],
                                lhsT=combT[:, k:k + 1, r0:r0 + mm],
                                rhs=wpc[:, k:k + 1, :],
                                start=(k == 0), stop=(k == KC_H - 1))
                        nc.scalar.activation(out=esb[:mm, mt, :vlim],
                                             in_=psp[:mm, :vlim],
                                             func=AF.Exp, scale=1.0 / WS,
                                             accum_out=zbuf[:mm, mt * NVC + vc:
                                                            mt * NVC + vc + 1])
                        if vc == 0:
                            nc.scalar.activation(out=cwn[:mm, mt:mt + 1],
                                                 in_=psp[:mm, COPY_ID:COPY_ID + 1],
                                                 func=AF.Exp, scale=1.0 / WS)
                    dma(out=e_dram[:, :, vc * VCH:vc * VCH + vlim],
                        in_=esb[:, :, :vlim])
                for mt, (r0, mm) in enumerate(mtiles):
                    zt = p2.tile([128, 1], F32, name="zt", tag="zt", bufs=2)
                    nc.vector.tensor_reduce(out=zt[:mm, :],
                                            in_=zbuf[:mm, mt * NVC:(mt + 1) * NVC],
                                            op=ALU.add, axis=mybir.AxisListType.X)
                    iz = p2.tile([128, 1], F32, name="iz", tag="zt", bufs=2)
                    nc.vector.reciprocal(out=iz[:mm, :], in_=zt[:mm, :])
                    nc.vector.tensor_tensor(out=cw[:mm, mt:mt + 1],
                                            in0=cwn[:mm, mt:mt + 1], in1=iz[:mm, :],
                                            op=ALU.mult)
                    omc = p2.tile([128, 1], F32, name="omc", tag="zt", bufs=2)
                    nc.vector.tensor_scalar(out=omc[:mm, :], in0=cw[:mm, mt:mt + 1],
                                            scalar1=-1.0, scalar2=1.0,
                                            op0=ALU.mult, op1=ALU.add)
                    nc.vector.tensor_tensor(out=spp[:mm, mt:mt + 1], in0=omc[:mm, :],
                                            in1=iz[:mm, :], op=ALU.mult)
                    nc.vector.tensor_scalar(out=ceps[:mm, mt:mt + 1],
                                            in0=cw[:mm, mt:mt + 1],
                                            scalar1=EPS, scalar2=None, op0=ALU.mult)
                for vc in range(NVC):
                    vlim = min(VCH, V - vc * VCH)
                    e2 = p2.tile([128, NMT, VCH], BF16, name="e2", tag="esb",
                                 bufs=2)
                    dma(out=e2[:, :, :vlim],
                        in_=e_dram[:, :, vc * VCH:vc * VCH + vlim])
                    outc = p2.tile([128, NMT, VCH], BF16, name="outc", tag="outc",
                                   bufs=2)
                    for mt, (r0, mm) in enumerate(mtiles):
                        if mm < 128:
                            nc.vector.memset(outc[:, mt, :], 0.0)
                        pscp = ps2.tile([128, VCH], F32, name="pscp", tag="psg",
                                        bufs=2)
                        nc.tensor.matmul(pscp[:mm, :vlim],
                                         lhsT=dsbA[:, r0:r0 + mm],
                                         rhs=ohA[:, vc * VCH:vc * VCH + vlim],
                                         start=True, stop=False)
                        nc.tensor.matmul(pscp[:mm, :vlim],
                                         lhsT=dsbB[:, r0:r0 + mm],
                                         rhs=ohB[:, vc * VCH:vc * VCH + vlim],
                                         start=False, stop=True)
                        nc.vector.tensor_scalar(out=pscp[:mm, :vlim],
                                                in0=pscp[:mm, :vlim],
                                                scalar1=cw[:mm, mt:mt + 1],
                                                scalar2=ceps[:mm, mt:mt + 1],
                                                op0=ALU.mult, op1=ALU.add)
                        ppf = p2.tile([128, VCH], F32, name="ppf", tag="ppf",
                                      bufs=2)
                        nc.vector.tensor_scalar(out=ppf[:mm, :vlim],
                                                in0=e2[:mm, mt, :vlim],
                                                scalar1=spp[:mm, mt:mt + 1],
                                                scalar2=None, op0=ALU.mult)
                        nc.vector.tensor_tensor(out=ppf[:mm, :vlim],
                                                in0=ppf[:mm, :vlim],
                                                in1=pscp[:mm, :vlim], op=ALU.add)
                        nc.scalar.activation(out=outc[:mm, mt, :vlim],
                                             in_=ppf[:mm, :vlim], func=AF.Ln)
                    dma(out=y_d[:, :, vc * VCH:vc * VCH + vlim],
                        in_=outc[:, :, :vlim])

    _split_wide_waits(nc)
    return nc


# ---------------------------------------------------------------- host prep
def _flatkT(w, dtype):
    """[K, N] -> flat SBUF image [128, (K//128)*N]"""
    K = w.shape[0]
    c = np.ascontiguousarray(w.reshape(K // 128, 128, -1))
    return np.ascontiguousarray(c.transpose(1, 0, 2).reshape(128, -1)).astype(dtype)


def _featmaj(x):
    """[BL, H] -> [128, KC_H*BL] feature-major (chunk-blocked transpose)"""
    xT = x.T  # [H, BL]
    return np.ascontiguousarray(
        xT.reshape(KC_H, 128, BL).transpose(1, 0, 2).reshape(128, KC_H * BL))


def prep_core_inputs(inputs, c, t_steps=T):
    ii = {k: np.asarray(v) for k, v in inputs.items()}
    Bc = list(range(c * BL, (c + 1) * BL))
    W_ih0 = ii["W_ih0"].astype(np.float32).copy()
    W_hh0 = ii["W_hh0"].astype(np.float32).copy()
    W_ih1 = ii["W_ih1"].astype(np.float32).copy()
    W_hh1 = ii["W_hh1"].astype(np.float32).copy()
    # fold tanh(g) = 2*sigmoid(2g)-1: double the g-gate weight rows
    for W in (W_ih0, W_hh0, W_ih1, W_hh1):
        W[2 * H:3 * H, :] *= 2.0
    Wc = ii["Wc"].astype(np.float32)
    Wp = ii["Wp"].astype(np.float32)
    Wk = ii["Wk"].astype(np.float32)
    enc = ii["enc_features"].astype(np.float32)
    embed = ii["embed"].astype(np.float32)
    rt, st = ii["ref_tokens"], ii["src_tokens"]

    Wfc = (W_ih0[:, E:] @ Wc).astype(np.float32)   # feed folded through Wc
    d = {}
    d["wfh"] = _flatkT(WS * Wfc[:, :H].T, nfp8)
    wfsT = np.ascontiguousarray((WS * Wfc[:, H:]).T)  # [H, G4]
    d["wfs"] = np.ascontiguousarray(
        wfsT.reshape(KC_H, 128, 2, 2048).transpose(2, 0, 1, 3)
        .reshape(16, 128, 2048)).astype(nfp8)
    d["wh08"] = _flatkT(WS * W_hh0.T, nfp8)
    d["wi18"] = _flatkT(WS * W_ih1.T, nfp8)
    d["wh18"] = _flatkT(WS * W_hh1.T, nfp8)
    d["we08"] = _flatkT(WS * W_ih0[:, :E].T, nfp8)
    d["wcb"] = _flatkT(WS * Wc.T, nfp8)
    # Wp^T padded, regrouped [vc][p][k*512+c]
    wpT = np.zeros((H, NVC * VCH), np.float32)
    wpT[:, :V] = Wp.T
    d["wpb"] = np.ascontiguousarray(
        (WS * wpT).reshape(KC_H, 128, NVC, VCH).transpose(2, 1, 0, 3)
        .reshape(NVC, 128, KC_H * VCH)).astype(nfp8)
    # Wk^T regrouped [mt][p][k*128+cc]
    d["wkT"] = np.ascontiguousarray(
        Wk.T.reshape(KC_H, 128, KC_H, 128).transpose(2, 1, 0, 3)
        .reshape(KC_H, 128, KC_H * 128)).astype(nbf16)
    # embed padded to 80 chunks, super-chunks of 10: [sc][p][j*E+c]
    embp = np.zeros((VKC8 * 128, E), np.float32)
    embp[:V] = embed
    d["emb8"] = np.ascontiguousarray(
        (WS * embp).reshape(16, 5, 128, E).transpose(0, 2, 1, 3)
        .reshape(16, 128, 5 * E)).astype(nfp8)
    # enc interleaved rows (s*4+b): tile A s<32, tile B s>=32
    encI = enc[:, Bc, :].reshape(S * BL, H)  # row s*BL+b
    d["encIA"] = np.ascontiguousarray(encI[0:128]).astype(nbf16)
    d["encIB"] = np.ascontiguousarray(encI[128:192]).astype(nbf16)
    # encT flat [p][k*192+(s,b)]
    encT = enc[:, Bc, :].transpose(2, 0, 1).reshape(H, S * BL)
    d["encT"] = _flatkT(encT, nbf16)
    # reftok replicated: col (t*BL + b)
    rtc = rt[:t_steps][:, Bc].astype(np.float32).reshape(t_steps * BL)
    d["reftok"] = np.tile(rtc[None, :], (128, 1)).astype(np.float32)
    d["vidx"] = (np.arange(128)[:, None]
                 + 128 * np.arange(VKC8)[None, :]).astype(np.float32)
    d["iota512"] = np.tile(np.arange(VCH, dtype=np.float32)[None, :], (128, 1))
    # srcsh [128, 2*NVC]: rows (s*4+b); tile 0: s<32, tile 1: s>=32
    stI = st[:, Bc].reshape(S * BL).astype(np.float32)  # row s*4+b
    srcsh = np.zeros((128, 2 * NVC), np.float32)
    for ch in range(NVC):
        srcsh[:, ch] = stI[0:128] - VCH * ch
        srcsh[0:64, NVC + ch] = stI[128:192] - VCH * ch
    d["srcsh"] = srcsh
    # pen_full [4, (s*4+b)]: row bp, col (s,b): -99999*mask if b==bp else -99999
    penf = np.full((BL, S * BL), -99999.0, np.float32)
    for bp in range(BL):
        penf[bp, bp::BL] = -99999.0 * (st[:, Bc[bp]] == PAD).astype(np.float32)
    d["pen"] = penf.astype(nbf16)
    h0 = ii["h0"].astype(np.float32)
    c0 = ii["c0"].astype(np.float32)
    d["h0T"] = _featmaj(h0[0][Bc]).astype(nfp8)
    d["h1T"] = _featmaj(h0[1][Bc]).astype(nfp8)
    d["c0T"] = _featmaj(c0[0][Bc]).astype(np.float32)
    d["c1T"] = _featmaj(c0[1][Bc]).astype(np.float32)
    d["ident4"] = np.eye(4, dtype=nbf16)
    # biases must be zero for this kernel (spec fill=zeros)
    for bn in ("bk", "bc", "bp", "b_ih0", "b_hh0", "b_ih1", "b_hh1"):
        assert np.abs(np.asarray(ii[bn])).max() == 0.0, f"nonzero bias {bn}"
    return d


def unpack_y(arr, t_steps=T):
    """[128, NMT, V] bf16 -> [t_steps, BL, V] f32"""
    NR = t_steps * BL
    nmt = arr.shape[1]
    flat = np.ascontiguousarray(
        np.asarray(arr).transpose(1, 0, 2)).reshape(nmt * 128, V)
    return flat[:NR].reshape(t_steps, BL, V).astype(np.float32)


def kernel(**inputs):
    t_steps = np.asarray(inputs["ref_tokens"]).shape[0]
    nc = build_program(t_steps)
    in_maps = [prep_core_inputs(inputs, c, t_steps) for c in range(NCORES)]
    res = run_bass_kernel_spmd(nc, in_maps, list(range(NCORES)))
    out = np.zeros((t_steps, B, V), np.float32)
    for c in range(NCORES):
        out[:, c * BL:(c + 1) * BL, :] = unpack_y(
            np.asarray(res.results[c]["y"]), t_steps)
    return out


if __name__ == "__main__":
    pass


# revision 21
# speedup vs baseline: 1.0133x; 1.0133x over previous
"""Trainium2 Bass kernel for nn_Decoder (LSTM decoder + attention + copy mechanism).

Strategy: pure batch-parallel across the 8 NeuronCores — each core runs the
full T=48-step recurrence and the vocab projection for its 4 batch elements,
with zero cross-core communication (this runtime exposes none).

The recurrence runs in feature-major layout: gates/hidden/cell live as
[feature-chunk(128-part), batch] tiles, the gate weights are the STATIONARY
matmul operand (lhsT, fp8 e3m4 resident in SBUF) and the batch-4 activations
stream as the 4-column moving operand, so a gate matmul costs 4 PE rows
instead of 512. tanh(g) is folded into one full-width sigmoid by pre-doubling
the g-gate weight rows on the host (tanh(x) = 2*sigmoid(2x)-1). The per-step
emission is software-pipelined: the next step's embedding/h0 gate matmuls and
layer-1 h1-part fill the PE while the current step's cell updates and
attention softmax run on Act/DVE/Pool.

All large DMA transfers are single instructions over host-prelaid [128, N]
images (HWDGE fixed cost ~650ns each makes many small DMAs expensive).

Self-contained: builds the Bass program, shards inputs on the host, runs via
run_bass_kernel_spmd on cores 0-7, reassembles the full [T, B, V] output.
"""
import sys

sys.path.insert(0, "/opt/trn_rl_repo")

import numpy as np
import ml_dtypes

import concourse.bass as bass
import concourse.mybir as mybir
import concourse.tile as tile
from concourse.bass_utils import run_bass_kernel_spmd

F32 = mybir.dt.float32
BF16 = mybir.dt.bfloat16
FP8 = mybir.dt.float8e3
I16 = mybir.dt.int16
AF = mybir.ActivationFunctionType
ALU = mybir.AluOpType

nbf16 = ml_dtypes.bfloat16
nfp8 = ml_dtypes.float8_e3m4
WS = 64.0                   # fp8 weight pre-scale (compensated in activations)
DR = mybir.MatmulPerfMode.DoubleRow

V, E, H = 10000, 512, 1024
T, S, B = 48, 48, 32
PAD, COPY_ID, EPS = 0, 1, 1e-7
NCORES = 8
BL = B // NCORES            # batch per core = 4
G4 = 4 * H                  # 4096 gate width
NVC = 20                    # vocab chunks of 512
VCH = 512
KC_E = E // 128             # 4
KC_H = H // 128             # 8
VKC8 = 80                   # padded vocab chunks (8 super-chunks of 10)

# psum column base per gate type (torch order i,f,g,o), laid out i|f|o|2g so
# the three sigmoids and doubled-g all go through one [0:128] sigmoid
_GCOL = {0: 0, 1: 32, 2: 96, 3: 64}


def _gcol(m):
    return _GCOL[m // 8] + (m % 8) * BL


# ---------------------------------------------------------------- wait split
def _split_wide_waits(nc):
    """walrus CTRL codegen accepts at most 1 sync-wait per instruction; move
    excess waits onto preceding NoOps on the same (in-order) engine."""
    for f in nc.m.functions:
        for bb in f.blocks:
            ins_list = list(bb.instructions)
            out = []
            changed = False
            for ins in ins_list:
                si = getattr(ins, "sync_info", None)
                waits = list(si.on_wait) if si is not None else []
                if len(waits) > 1:
                    excess, keep = waits[:-1], waits[-1:]
                    for w in excess:
                        nop = mybir.InstNoOp(
                            name=f"I-{nc.next_id()}",
                            opcode="NoOp",
                            engine=ins.engine,
                            debug=ins.debug,
                            ins=[],
                            outs=[],
                            sync_info=mybir.SyncInfo(on_wait=[w], on_update=[]),
                        )
                        try:
                            nc.register_instruction(nop, overwrite=True)
                        except Exception:
                            pass
                        out.append(nop)
                        changed = True
                    si.on_wait = keep
                    ins.sync_info = si
                out.append(ins)
            if changed:
                try:
                    bb.instructions = out
                except Exception:
                    bb.instructions.clear()
                    bb.instructions.extend(out)


# ---------------------------------------------------------------- program
def build_program(t_steps=T):
    nc = bass.Bass("TRN2")
    dp = nc.declare_dram_parameter

    NR = t_steps * BL
    mtiles = [(r0, min(128, NR - r0)) for r0 in range(0, NR, 128)]
    NMT = len(mtiles)

    # all weight images are host-prelaid as a flat [128, N] SBUF image
    wfh_d = dp("wfh", [128, KC_H * G4], FP8, isOutput=False)  # (Wf@Wc)[:, :H]^T
    wfs_d = dp("wfs", [16, 128, 2048], FP8, isOutput=False)   # (Wf@Wc)[:, H:]^T
    wh08_d = dp("wh08", [128, KC_H * G4], FP8, isOutput=False)   # W_hh0^T
    wi18_d = dp("wi18", [128, KC_H * G4], FP8, isOutput=False)   # W_ih1^T
    wh18_d = dp("wh18", [128, KC_H * G4], FP8, isOutput=False)   # W_hh1^T
    we08_d = dp("we08", [128, KC_E * G4], FP8, isOutput=False)   # W_ih0[:,:E]^T
    wcb_d = dp("wcb", [128, 2 * KC_H * H], FP8, isOutput=False)  # Wc^T
    wpb_d = dp("wpb", [NVC, 128, KC_H * VCH], FP8, isOutput=False)  # Wp^T by vc
    wkT_d = dp("wkT", [KC_H, 128, KC_H * 128], BF16, isOutput=False)  # Wk^T by mt
    emb_d = dp("emb8", [16, 128, 5 * E], FP8, isOutput=False)   # embed^T chunks
    encIA_d = dp("encIA", [128, H], BF16, isOutput=False)  # enc rows (s*4+b), s<32
    encIB_d = dp("encIB", [64, H], BF16, isOutput=False)   # s in 32..47
    encT_d = dp("encT", [128, KC_H * BL * S], BF16, isOutput=False)
    reftok_d = dp("reftok", [128, NR], F32, isOutput=False)
    vidx_d = dp("vidx", [128, VKC8], F32, isOutput=False)       # p + 128*ch
    iota512_d = dp("iota512", [128, VCH], F32, isOutput=False)
    srcsh_d = dp("srcsh", [128, 2 * NVC], F32, isOutput=False)  # rows (s*4+b)
    pen_d = dp("pen", [BL, S * BL], BF16, isOutput=False)       # penalty incl mask
    h0T_d = dp("h0T", [128, KC_H * BL], FP8, isOutput=False)
    h1T_d = dp("h1T", [128, KC_H * BL], FP8, isOutput=False)
    c0T_d = dp("c0T", [128, KC_H * BL], F32, isOutput=False)
    c1T_d = dp("c1T", [128, KC_H * BL], F32, isOutput=False)
    ident4_d = dp("ident4", [4, 4], BF16, isOutput=False)

    y_d = dp("y", [128, NMT, V], BF16, isOutput=True)  # host reorders + casts

    with tile.TileContext(nc) as tc:
        with tc.tile_pool(name="wres", bufs=1) as wpool, \
             tc.tile_pool(name="dram", bufs=1, space="DRAM") as dpool:

            e_dram = dpool.tile([128, NMT, NVC * VCH], BF16, name="e_dram")

            dma = nc.sync.dma_start

            # ---- outer-resident (survive into phase 2)
            combT = wpool.tile([128, KC_H, NR], FP8, name="combT")
            dsbA = wpool.tile([128, NR], BF16, name="dsbA")
            dsbB = wpool.tile([64, NR], BF16, name="dsbB")
            zbuf = wpool.tile([128, 2 * NVC], F32, name="zbuf")
            cwn = wpool.tile([128, 2], F32, name="cwn")
            cw = wpool.tile([128, 2], F32, name="cw")
            spp = wpool.tile([128, 2], F32, name="spp")
            ceps = wpool.tile([128, 2], F32, name="ceps")
            ident4 = wpool.tile([4, 4], BF16, name="ident4")
            srcsh = wpool.tile([128, 2 * NVC], F32, name="srcsh")
            iota512 = wpool.tile([128, VCH], F32, name="iota512")
            dma(out=ident4[:], in_=ident4_d[:])
            dma(out=srcsh[:], in_=srcsh_d[:])
            dma(out=iota512[:], in_=iota512_d[:])

            # ======== phases 0+1 (scoped pool; weights freed before phase 2)
            with tc.tile_pool(name="ph01", bufs=1) as p1:
                wfh = p1.tile([128, KC_H, G4], FP8, name="wfh")
                QA = p1.tile([128, 32 * 128], BF16, name="QA")
                QB = p1.tile([64, 32 * 128], BF16, name="QB")
                wh0 = p1.tile([128, KC_H, G4], FP8, name="wh0")
                wi1 = p1.tile([128, KC_H, G4], FP8, name="wi1")
                wh1 = p1.tile([128, KC_H, G4], FP8, name="wh1")
                we0 = p1.tile([128, KC_E, G4], FP8, name="we0")
                wcb = p1.tile([128, 2 * KC_H, H], FP8, name="wcb")
                XeT = p1.tile([128, KC_E, NR], FP8, name="XeT")
                attKT = p1.tile([128, KC_H, BL * S], FP8, name="attKT")
                encIA = p1.tile([128, H], BF16, name="encIA")
                encIB = p1.tile([64, H], BF16, name="encIB")
                pen = p1.tile([BL, S * BL], BF16, name="pen")
                h0T = p1.tile([128, KC_H, BL], FP8, name="h0T")
                h1T = p1.tile([128, KC_H, BL], FP8, name="h1T")
                c0T = p1.tile([128, KC_H * BL], F32, name="c0T")
                c1T = p1.tile([128, KC_H * BL], F32, name="c1T")
                combT0 = p1.tile([128, KC_H, BL], FP8, name="combT0")
                sumT = p1.tile([128, KC_H, BL], FP8, name="sumT")

                # small state first, then weights in first-use order
                dma(out=h0T[:], in_=h0T_d[:])
                dma(out=h1T[:], in_=h1T_d[:])
                dma(out=c0T[:], in_=c0T_d[:])
                dma(out=c1T[:], in_=c1T_d[:])
                dma(out=pen[:], in_=pen_d[:])
                dma(out=encIA[:], in_=encIA_d[:])
                dma(out=encIB[:], in_=encIB_d[:])
                nc.vector.memset(combT0[:], 0.0)

                # ---- phase 0a: X_embT = embed^T @ onehot(ref_tokens)
                with tc.tile_pool(name="ph0", bufs=1) as p0:
                    with tc.tile_pool(name="ph0a", bufs=1) as p0a, \
                         tc.tile_pool(name="ps0a", bufs=1, space="PSUM") as ps0a:
                        reftok = p0a.tile([128, NR], F32, name="reftok")
                        vidx = p0a.tile([128, VKC8], F32, name="vidx")
                        dma(out=reftok[:], in_=reftok_d[:])
                        dma(out=vidx[:], in_=vidx_d[:])
                        psX = [ps0a.tile([128, NR], F32, name=f"psX{m}",
                                         tag=f"psX{m}", bufs=1)
                               for m in range(KC_E)]
                        for sc in range(16):
                            emb8 = p0a.tile([128, 5 * E], FP8, name="emb8",
                                            tag="emb8", bufs=2)
                            dma(out=emb8[:], in_=emb_d[sc])
                            for j in range(5):
                                ch = sc * 5 + j
                                oref = p0a.tile([128, NR], BF16, name="oref",
                                                tag="oref", bufs=2)
                                nc.vector.tensor_scalar(
                                    out=oref[:], in0=reftok[:],
                                    scalar1=vidx[:, ch:ch + 1],
                                    scalar2=None, op0=ALU.is_equal)
                                for m in range(KC_E):
                                    nc.tensor.matmul(
                                        psX[m][:],
                                        lhsT=emb8[:, j * E + m * 128:
                                                  j * E + (m + 1) * 128],
                                        rhs=oref[:], start=(ch == 0),
                                        stop=(ch == VKC8 - 1))
                        for m in range(KC_E):
                            nc.vector.tensor_scalar(out=XeT[:, m, :],
                                                    in0=psX[m][:],
                                                    scalar1=1.0 / WS,
                                                    scalar2=None, op0=ALU.mult)

                    # gate weights (one DMA each, first-use order)
                    dma(out=wh0[:], in_=wh08_d[:])
                    dma(out=we0[:], in_=we08_d[:])
                    dma(out=wh1[:], in_=wh18_d[:])
                    dma(out=wi1[:], in_=wi18_d[:])

                    encTs = p0.tile([128, KC_H * BL * S], BF16, name="encTs")
                    dma(out=encTs[:], in_=encT_d[:])

                    # ---- phase 0b: Q^T = WS * enc @ Wfc_s^T  (two jc passes)
                    with tc.tile_pool(name="ps0q", bufs=1, space="PSUM") as ps0q:
                        for ph in range(2):
                            psq = [ps0q.tile([128, VCH], F32, name=f"psq{i}",
                                             tag=f"psq{i}", bufs=1)
                                   for i in range(8)]
                            qtiles = [(0, 128), (128, 64)]
                            for k in range(KC_H):
                                wfsk = p0.tile([128, 2048], FP8, name="wfsk",
                                               tag="wfsk", bufs=2)
                                dma(out=wfsk[:], in_=wfs_d[ph * KC_H + k])
                                for mt2, (r0, mm) in enumerate(qtiles):
                                    for jc in range(4):
                                        nc.tensor.matmul(
                                            psq[mt2 * 4 + jc][:mm, :],
                                            lhsT=encTs[:, k * BL * S + r0:
                                                       k * BL * S + r0 + mm],
                                            rhs=wfsk[:, jc * VCH:(jc + 1) * VCH],
                                            start=(k == 0), stop=(k == KC_H - 1))
                            for mt2, (r0, mm) in enumerate(qtiles):
                                qdst = QA if mt2 == 0 else QB
                                for jc in range(4):
                                    nc.vector.tensor_copy(
                                        out=qdst[:mm, (ph * 4 + jc) * VCH:
                                                 (ph * 4 + jc + 1) * VCH],
                                        in_=psq[mt2 * 4 + jc][:mm, :])

                    dma(out=wfh[:], in_=wfh_d[:])

                    # ---- phase 0c: att_keyT = Wk @ enc^T
                    with tc.tile_pool(name="ps0c", bufs=1, space="PSUM") as ps0c:
                        for mt in range(KC_H):
                            wkmt = p0.tile([128, KC_H * 128], BF16, name="wkmt",
                                           tag="wkmt", bufs=1)
                            dma(out=wkmt[:], in_=wkT_d[mt])
                            psa = ps0c.tile([128, BL * S], F32, name="psa",
                                            tag="psa", bufs=2)
                            for k in range(KC_H):
                                nc.tensor.matmul(
                                    psa[:], lhsT=wkmt[:, k * 128:(k + 1) * 128],
                                    rhs=encTs[:, k * BL * S:(k + 1) * BL * S],
                                    start=(k == 0), stop=(k == KC_H - 1))
                            nc.vector.tensor_copy(out=attKT[:, mt, :], in_=psa[:])

                    dma(out=wcb[:], in_=wcb_d[:])

                # ======== phase 1: software-pipelined recurrence
                SIG, TANH = AF.Sigmoid, AF.Tanh
                with tc.tile_pool(name="gps", bufs=3, space="PSUM") as gps, \
                     tc.tile_pool(name="sps", bufs=1, space="PSUM") as sps:

                    g0t = {}
                    g1t = {}

                    def getg(d, t):
                        if t not in d:
                            d[t] = gps.tile([128, 128], F32, name="g", tag="g",
                                            bufs=3)
                        return d[t]

                    def drmm(g, w, rhs3, kp, m, start, stop):
                        c = _gcol(m)
                        nc.tensor.matmul(
                            g[:, c:c + BL],
                            lhsT=w[:, 2 * kp:2 * kp + 1, m * 128:(m + 1) * 128],
                            rhs=rhs3[:, 0:1, :], start=start, stop=False)
                        nc.tensor.matmul(
                            g[:, c:c + BL],
                            lhsT=w[:, 2 * kp + 1:2 * kp + 2, m * 128:(m + 1) * 128],
                            rhs=rhs3[:, 1:2, :], start=False, stop=stop)

                    def emit_A(t, part):
                        g0 = getg(g0t, t)
                        if part == "xe":
                            w, kk = we0, KC_E // 2
                            rf = lambda kp: XeT[:, 2 * kp:2 * kp + 2,
                                                t * BL:(t + 1) * BL]
                        elif part == "h0":
                            w, kk = wh0, KC_H // 2
                            rf = lambda kp: h0T[:, 2 * kp:2 * kp + 2, :]
                        else:  # "fh": Wfc_h @ h1^{t-1}
                            w, kk = wfh, KC_H // 2
                            rf = lambda kp: h1T[:, 2 * kp:2 * kp + 2, :]
                        first = part == "xe"
                        last = part == "h0" and t == 0
                        for kp in range(kk):
                            rhs = rf(kp)
                            for m in range(32):
                                drmm(g0, w, rhs, kp, m,
                                     first and kp == 0 and m == 0,
                                     last and kp == kk - 1 and m == 31)

                    def emit_Q(t, tdist):  # gates0 += Q^T @ dist^{tdist}
                        g0 = getg(g0t, t)
                        for m in range(32):
                            c = _gcol(m)
                            nc.tensor.matmul(
                                g0[:, c:c + BL], lhsT=QA[:, m * 128:(m + 1) * 128],
                                rhs=dsbA[:, tdist * BL:(tdist + 1) * BL],
                                start=False, stop=False)
                            nc.tensor.matmul(
                                g0[:, c:c + BL], lhsT=QB[:, m * 128:(m + 1) * 128],
                                rhs=dsbB[:, tdist * BL:(tdist + 1) * BL],
                                start=False, stop=(m == 31))

                    def emit_B(t):  # L1 gates, wh1 @ h1^{t-1}
                        g1 = getg(g1t, t)
                        for kp in range(KC_H // 2):
                            rhs = h1T[:, 2 * kp:2 * kp + 2, :]
                            for m in range(32):
                                drmm(g1, wh1, rhs, kp, m, kp == 0 and m == 0,
                                     False)

                    def emit_C(t):  # L1 gates, wi1 @ h0^t
                        g1 = getg(g1t, t)
                        for kp in range(KC_H // 2):
                            rhs = h0T[:, 2 * kp:2 * kp + 2, :]
                            for m in range(32):
                                drmm(g1, wi1, rhs, kp, m, False,
                                     kp == KC_H // 2 - 1 and m == 31)

                    def emit_cell(t, layer):
                        g = (g0t if layer == 0 else g1t).pop(t)
                        cT = c0T if layer == 0 else c1T
                        hT = h0T if layer == 0 else h1T
                        gs = p1.tile([128, 128], F32, name="gs", tag="gs", bufs=2)
                        nc.scalar.activation(out=gs[:], in_=g[:], func=SIG,
                                             scale=1.0 / WS)
                        gg = p1.tile([128, 32], F32, name="gg", tag="gg", bufs=2)
                        nc.vector.tensor_scalar(out=gg[:], in0=gs[:, 96:128],
                                                scalar1=2.0, scalar2=-1.0,
                                                op0=ALU.mult, op1=ALU.add)
                        u = p1.tile([128, 32], F32, name="u", tag="u", bufs=2)
                        nc.vector.tensor_tensor(out=u[:], in0=gs[:, 0:32],
                                                in1=gg[:], op=ALU.mult)
                        nc.vector.tensor_tensor(out=cT[:], in0=gs[:, 32:64],
                                                in1=cT[:], op=ALU.mult)
                        nc.vector.tensor_tensor(out=cT[:], in0=cT[:], in1=u[:],
                                                op=ALU.add)
                        th = p1.tile([128, 32], F32, name="th", tag="th", bufs=2)
                        nc.scalar.activation(out=th[:], in_=cT[:], func=TANH)
                        nc.vector.tensor_tensor(out=hT[:, :, :], in0=gs[:, 64:96],
                                                in1=th[:], op=ALU.mult)

                    def emit_att(t):
                        pss = sps.tile([BL, BL * S], F32, name="pss", tag="pss",
                                       bufs=1)
                        for k in range(KC_H):
                            nc.tensor.matmul(
                                pss[:], lhsT=h1T[:, k:k + 1, :],
                                rhs=attKT[:, k:k + 1, :],
                                start=(k == 0), stop=False)
                        nc.tensor.matmul(pss[:], lhsT=ident4[:], rhs=pen[:],
                                         start=False, stop=True)
                        ssum = p1.tile([BL, 1], F32, name="ssum", tag="ssum",
                                       bufs=2)
                        dstc = p1.tile([BL, S * BL], F32, name="dstc", tag="dstc",
                                       bufs=2)
                        nc.scalar.activation(out=dstc[:], in_=pss[:], func=AF.Exp,
                                             accum_out=ssum[:])
                        rs = p1.tile([BL, 1], F32, name="rs", tag="rs", bufs=2)
                        nc.vector.reciprocal(out=rs[:], in_=ssum[:])
                        dstb = p1.tile([BL, S * BL], BF16, name="dstb", tag="dstb",
                                       bufs=2)
                        nc.vector.tensor_scalar(out=dstb[:], in0=dstc[:],
                                                scalar1=rs[:], scalar2=None,
                                                op0=ALU.mult)
                        return dstb

                    def emit_sumcomb(t, dstb):
                        psDA = sps.tile([128, BL], BF16, name="psDA", tag="psT",
                                        bufs=1)
                        nc.tensor.transpose(psDA[:], dstb[:, 0:128], ident4[:])
                        nc.vector.tensor_copy(out=dsbA[:, t * BL:(t + 1) * BL],
                                              in_=psDA[:])
                        psDB = sps.tile([64, BL], BF16, name="psDB", tag="psTB",
                                        bufs=1)
                        nc.tensor.transpose(psDB[:], dstb[:, 128:192], ident4[:])
                        nc.vector.tensor_copy(out=dsbB[:, t * BL:(t + 1) * BL],
                                              in_=psDB[:])
                        pssu = sps.tile([128, KC_H * BL], F32, name="pssu",
                                        tag="pssu", bufs=1)
                        for j in range(KC_H):
                            nc.tensor.matmul(
                                pssu[:, j * BL:(j + 1) * BL],
                                lhsT=encIA[:, j * 128:(j + 1) * 128],
                                rhs=dsbA[:, t * BL:(t + 1) * BL],
                                start=(j == 0), stop=False)
                            nc.tensor.matmul(
                                pssu[:, j * BL:(j + 1) * BL],
                                lhsT=encIB[:, j * 128:(j + 1) * 128],
                                rhs=dsbB[:, t * BL:(t + 1) * BL],
                                start=False, stop=(j == KC_H - 1))
                        nc.vector.tensor_copy(out=sumT[:, :, :], in_=pssu[:])
                        psc = sps.tile([128, KC_H * BL], F32, name="psc",
                                       tag="psc", bufs=1)
                        for k in range(KC_H):
                            rhs = h1T[:, k:k + 1, :]
                            for mcc in range(KC_H):
                                nc.tensor.matmul(
                                    psc[:, mcc * BL:(mcc + 1) * BL],
                                    lhsT=wcb[:, k:k + 1,
                                             mcc * 128:(mcc + 1) * 128],
                                    rhs=rhs, start=(k == 0 and mcc == 0),
                                    stop=False)
                        for k in range(KC_H):
                            rhs = sumT[:, k:k + 1, :]
                            for mcc in range(KC_H):
                                nc.tensor.matmul(
                                    psc[:, mcc * BL:(mcc + 1) * BL],
                                    lhsT=wcb[:, KC_H + k:KC_H + k + 1,
                                             mcc * 128:(mcc + 1) * 128],
                                    rhs=rhs, start=False,
                                    stop=(k == KC_H - 1 and mcc == KC_H - 1))
                        nc.vector.tensor_scalar(
                            out=combT[:, :, t * BL:(t + 1) * BL], in0=psc[:],
                            scalar1=1.0 / WS, scalar2=None, op0=ALU.mult)

                    for t in range(t_steps):
                        if t == 0:
                            emit_A(0, "xe")
                            emit_A(0, "h0")
                            emit_B(0)
                        emit_cell(t, 0)
                        emit_C(t)
                        if t + 1 < t_steps:
                            emit_A(t + 1, "xe")
                            emit_A(t + 1, "h0")
                        emit_cell(t, 1)
                        if t + 1 < t_steps:
                            emit_A(t + 1, "fh")
                        dstb = emit_att(t)
                        if t + 1 < t_steps:
                            emit_B(t + 1)
                        emit_sumcomb(t, dstb)
                        if t + 1 < t_steps:
                            emit_Q(t + 1, t)

            # ======== phase 2 (own pools; vc outer so Wp streams once)
            with tc.tile_pool(name="ph2", bufs=1) as p2, \
                 tc.tile_pool(name="ps2", bufs=2, space="PSUM") as ps2:
                # one-hot tiles, SBUF-resident fp8 (generated here on DVE)
                ohA = p2.tile([128, NVC * VCH], BF16, name="ohA")
                ohB = p2.tile([64, NVC * VCH], BF16, name="ohB")
                for tl, nrow, oh in ((0, 128, ohA), (1, 64, ohB)):
                    for ch in range(NVC):
                        nc.vector.tensor_scalar(
                            out=oh[:nrow, ch * VCH:(ch + 1) * VCH],
                            in0=iota512[:nrow, :],
                            scalar1=srcsh[:nrow, tl * NVC + ch:tl * NVC + ch + 1],
                            scalar2=None, op0=ALU.is_equal)

                for vc in range(NVC):
                    vlim = min(VCH, V - vc * VCH)
                    wpc = p2.tile([128, KC_H, VCH], FP8, name="wpc", tag="wpc",
                                  bufs=2)
                    dma(out=wpc[:], in_=wpb_d[vc])
                    esb = p2.tile([128, NMT, VCH], BF16, name="esb", tag="esb",
                                  bufs=2)
                    for mt, (r0, mm) in enumerate(mtiles):
                        if mm < 128:
                            nc.vector.memset(esb[:, mt, :], 0.0)
                        psp = ps2.tile([128, VCH], F32, name="psp", tag="psg",
                                       bufs=2)
                        for k in range(KC_H):
                            nc.tensor.matmul(
                                psp[:mm, :],
                                lhsT=combT[:, k:k + 1, r0:r0 + mm],
                                rhs=wpc[:, k:k + 1, :],
                                start=(k == 0), stop=(k == KC_H - 1))
                        nc.scalar.activation(out=esb[:mm, mt, :vlim],
                                             in_=psp[:mm, :vlim],
                                             func=AF.Exp, scale=1.0 / WS,
                                             accum_out=zbuf[:mm, mt * NVC + vc:
                                                            mt * NVC + vc + 1])
                        if vc == 0:
                            nc.scalar.activation(out=cwn[:mm, mt:mt + 1],
                                                 in_=psp[:mm, COPY_ID:COPY_ID + 1],
                                                 func=AF.Exp, scale=1.0 / WS)
                    dma(out=e_dram[:, :, vc * VCH:vc * VCH + vlim],
                        in_=esb[:, :, :vlim])
                for mt, (r0, mm) in enumerate(mtiles):
                    zt = p2.tile([128, 1], F32, name="zt", tag="zt", bufs=2)
                    nc.vector.tensor_reduce(out=zt[:mm, :],
                                            in_=zbuf[:mm, mt * NVC:(mt + 1) * NVC],
                                            op=ALU.add, axis=mybir.AxisListType.X)
                    iz = p2.tile([128, 1], F32, name="iz", tag="zt", bufs=2)
                    nc.vector.reciprocal(out=iz[:mm, :], in_=zt[:mm, :])
                    nc.vector.tensor_tensor(out=cw[:mm, mt:mt + 1],
                                            in0=cwn[:mm, mt:mt + 1], in1=iz[:mm, :],
                                            op=ALU.mult)
                    omc = p2.tile([128, 1], F32, name="omc", tag="zt", bufs=2)
                    nc.vector.tensor_scalar(out=omc[:mm, :], in0=cw[:mm, mt:mt + 1],
                                            scalar1=-1.0, scalar2=1.0,
                                            op0=ALU.mult, op1=ALU.add)
                    nc.vector.tensor_tensor(out=spp[:mm, mt:mt + 1], in0=omc[:mm, :],
                                            in1=iz[:mm, :], op=ALU.mult)
                    nc.vector.tensor_scalar(out=ceps[:mm, mt:mt + 1],
                                            in0=cw[:mm, mt:mt + 1],
                                            scalar1=EPS, scalar2=None, op0=ALU.mult)
                for vc in range(NVC):
                    vlim = min(VCH, V - vc * VCH)
                    e2 = p2.tile([128, NMT, VCH], BF16, name="e2", tag="esb",
                                 bufs=2)
                    dma(out=e2[:, :, :vlim],
                        in_=e_dram[:, :, vc * VCH:vc * VCH + vlim])
                    outc = p2.tile([128, NMT, VCH], BF16, name="outc", tag="outc",
                                   bufs=2)
                    for mt, (r0, mm) in enumerate(mtiles):
                        if mm < 128:
                            nc.vector.memset(outc[:, mt, :], 0.0)
                        pscp = ps2.tile([128, VCH], F32, name="pscp", tag="psg",
                                        bufs=2)
                        nc.tensor.matmul(pscp[:mm, :vlim],
                                         lhsT=dsbA[:, r0:r0 + mm],
                                         rhs=ohA[:, vc * VCH:vc * VCH + vlim],
                                         start=True, stop=False)
                        nc.tensor.matmul(pscp[:mm, :vlim],
                                         lhsT=dsbB[:, r0:r0 + mm],
                                         rhs=ohB[:, vc * VCH:vc * VCH + vlim],
                                         start=False, stop=True)
                        nc.vector.tensor_scalar(out=pscp[:mm, :vlim],
                                                in0=pscp[:mm, :vlim],
                                                scalar1=cw[:mm, mt:mt + 1],
                                                scalar2=ceps[:mm, mt:mt + 1],
                                                op0=ALU.mult, op1=ALU.add)
                        ppf = p2.tile([128, VCH], F32, name="ppf", tag="ppf",
                                      bufs=2)
                        nc.vector.tensor_scalar(out=ppf[:mm, :vlim],
                                                in0=e2[:mm, mt, :vlim],
                                                scalar1=spp[:mm, mt:mt + 1],
                                                scalar2=None, op0=ALU.mult)
                        nc.vector.tensor_tensor(out=ppf[:mm, :vlim],
                                                in0=ppf[:mm, :vlim],
                                                in1=pscp[:mm, :vlim], op=ALU.add)
                        nc.scalar.activation(out=outc[:mm, mt, :vlim],
                                             in_=ppf[:mm, :vlim], func=AF.Ln)
                    dma(out=y_d[:, :, vc * VCH:vc * VCH + vlim],
                        in_=outc[:, :, :vlim])

    _split_wide_waits(nc)
    return nc


# ---------------------------------------------------------------- host prep
def _flatkT(w, dtype):
    """[K, N] -> flat SBUF image [128, (K//128)*N]"""
    K = w.shape[0]
    c = np.ascontiguousarray(w.reshape(K // 128, 128, -1))
    return np.ascontiguousarray(c.transpose(1, 0, 2).reshape(128, -1)).astype(dtype)


def _featmaj(x):
    """[BL, H] -> [128, KC_H*BL] feature-major (chunk-blocked transpose)"""
    xT = x.T  # [H, BL]
    return np.ascontiguousarray(
        xT.reshape(KC_H, 128, BL).transpose(1, 0, 2).reshape(128, KC_H * BL))


def prep_core_inputs(inputs, c, t_steps=T):
    ii = {k: np.asarray(v) for k, v in inputs.items()}
    Bc = list(range(c * BL, (c + 1) * BL))
    W_ih0 = ii["W_ih0"].astype(np.float32).copy()
    W_hh0 = ii["W_hh0"].astype(np.float32).copy()
    W_ih1 = ii["W_ih1"].astype(np.float32).copy()
    W_hh1 = ii["W_hh1"].astype(np.float32).copy()
    # fold tanh(g) = 2*sigmoid(2g)-1: double the g-gate weight rows
    for W in (W_ih0, W_hh0, W_ih1, W_hh1):
        W[2 * H:3 * H, :] *= 2.0
    Wc = ii["Wc"].astype(np.float32)
    Wp = ii["Wp"].astype(np.float32)
    Wk = ii["Wk"].astype(np.float32)
    enc = ii["enc_features"].astype(np.float32)
    embed = ii["embed"].astype(np.float32)
    rt, st = ii["ref_tokens"], ii["src_tokens"]

    Wfc = (W_ih0[:, E:] @ Wc).astype(np.float32)   # feed folded through Wc
    d = {}
    d["wfh"] = _flatkT(WS * Wfc[:, :H].T, nfp8)
    wfsT = np.ascontiguousarray((WS * Wfc[:, H:]).T)  # [H, G4]
    d["wfs"] = np.ascontiguousarray(
        wfsT.reshape(KC_H, 128, 2, 2048).transpose(2, 0, 1, 3)
        .reshape(16, 128, 2048)).astype(nfp8)
    d["wh08"] = _flatkT(WS * W_hh0.T, nfp8)
    d["wi18"] = _flatkT(WS * W_ih1.T, nfp8)
    d["wh18"] = _flatkT(WS * W_hh1.T, nfp8)
    d["we08"] = _flatkT(WS * W_ih0[:, :E].T, nfp8)
    d["wcb"] = _flatkT(WS * Wc.T, nfp8)
    # Wp^T padded, regrouped [vc][p][k*512+c]
    wpT = np.zeros((H, NVC * VCH), np.float32)
    wpT[:, :V] = Wp.T
    d["wpb"] = np.ascontiguousarray(
        (WS * wpT).reshape(KC_H, 128, NVC, VCH).transpose(2, 1, 0, 3)
        .reshape(NVC, 128, KC_H * VCH)).astype(nfp8)
    # Wk^T regrouped [mt][p][k*128+cc]
    d["wkT"] = np.ascontiguousarray(
        Wk.T.reshape(KC_H, 128, KC_H, 128).transpose(2, 1, 0, 3)
        .reshape(KC_H, 128, KC_H * 128)).astype(nbf16)
    # embed padded to 80 chunks, super-chunks of 10: [sc][p][j*E+c]
    embp = np.zeros((VKC8 * 128, E), np.float32)
    embp[:V] = embed
    d["emb8"] = np.ascontiguousarray(
        (WS * embp).reshape(16, 5, 128, E).transpose(0, 2, 1, 3)
        .reshape(16, 128, 5 * E)).astype(nfp8)
    # enc interleaved rows (s*4+b): tile A s<32, tile B s>=32
    encI = enc[:, Bc, :].reshape(S * BL, H)  # row s*BL+b
    d["encIA"] = np.ascontiguousarray(encI[0:128]).astype(nbf16)
    d["encIB"] = np.ascontiguousarray(encI[128:192]).astype(nbf16)
    # encT flat [p][k*192+(s,b)]
    encT = enc[:, Bc, :].transpose(2, 0, 1).reshape(H, S * BL)
    d["encT"] = _flatkT(encT, nbf16)
    # reftok replicated: col (t*BL + b)
    rtc = rt[:t_steps][:, Bc].astype(np.float32).reshape(t_steps * BL)
    d["reftok"] = np.tile(rtc[None, :], (128, 1)).astype(np.float32)
    d["vidx"] = (np.arange(128)[:, None]
                 + 128 * np.arange(VKC8)[None, :]).astype(np.float32)
    d["iota512"] = np.tile(np.arange(VCH, dtype=np.float32)[None, :], (128, 1))
    # srcsh [128, 2*NVC]: rows (s*4+b); tile 0: s<32, tile 1: s>=32
    stI = st[:, Bc].reshape(S * BL).astype(np.float32)  # row s*4+b
    srcsh = np.zeros((128, 2 * NVC), np.float32)
    for ch in range(NVC):
        srcsh[:, ch] = stI[0:128] - VCH * ch
        srcsh[0:64, NVC + ch] = stI[128:192] - VCH * ch
    d["srcsh"] = srcsh
    # pen_full [4, (s*4+b)]: row bp, col (s,b): -99999*mask if b==bp else -99999
    penf = np.full((BL, S * BL), -99999.0, np.float32)
    for bp in range(BL):
        penf[bp, bp::BL] = -99999.0 * (st[:, Bc[bp]] == PAD).astype(np.float32)
    d["pen"] = penf.astype(nbf16)
    h0 = ii["h0"].astype(np.float32)
    c0 = ii["c0"].astype(np.float32)
    d["h0T"] = _featmaj(h0[0][Bc]).astype(nfp8)
    d["h1T"] = _featmaj(h0[1][Bc]).astype(nfp8)
    d["c0T"] = _featmaj(c0[0][Bc]).astype(np.float32)
    d["c1T"] = _featmaj(c0[1][Bc]).astype(np.float32)
    d["ident4"] = np.eye(4, dtype=nbf16)
    # biases must be zero for this kernel (spec fill=zeros)
    for bn in ("bk", "bc", "bp", "b_ih0", "b_hh0", "b_ih1", "b_hh1"):
        assert np.abs(np.asarray(ii[bn])).max() == 0.0, f"nonzero bias {bn}"
    return d


def unpack_y(arr, t_steps=T):
    """[128, NMT, V] bf16 -> [t_steps, BL, V] f32"""
    NR = t_steps * BL
    nmt = arr.shape[1]
    flat = np.ascontiguousarray(
        np.asarray(arr).transpose(1, 0, 2)).reshape(nmt * 128, V)
    return flat[:NR].reshape(t_steps, BL, V).astype(np.float32)


def kernel(**inputs):
    t_steps = np.asarray(inputs["ref_tokens"]).shape[0]
    nc = build_program(t_steps)
    in_maps = [prep_core_inputs(inputs, c, t_steps) for c in range(NCORES)]
    res = run_bass_kernel_spmd(nc, in_maps, list(range(NCORES)))
    out = np.zeros((t_steps, B, V), np.float32)
    for c in range(NCORES):
        out[:, c * BL:(c + 1) * BL, :] = unpack_y(
            np.asarray(res.results[c]["y"]), t_steps)
    return out


if __name__ == "__main__":
    pass


# revision 22
# speedup vs baseline: 1.0217x; 1.0082x over previous
"""Trainium2 Bass kernel for nn_Decoder (LSTM decoder + attention + copy mechanism).

Strategy: pure batch-parallel across the 8 NeuronCores — each core runs the
full T=48-step recurrence and the vocab projection for its 4 batch elements,
with zero cross-core communication (this runtime exposes none).

The recurrence runs in feature-major layout: gates/hidden/cell live as
[feature-chunk(128-part), batch] tiles, the gate weights are the STATIONARY
matmul operand (lhsT, fp8 e3m4 resident in SBUF) and the batch-4 activations
stream as the 4-column moving operand, so a gate matmul costs 4 PE rows
instead of 512. tanh(g) is folded into one full-width sigmoid by pre-doubling
the g-gate weight rows on the host (tanh(x) = 2*sigmoid(2x)-1). The per-step
emission is software-pipelined: the next step's embedding/h0 gate matmuls and
layer-1 h1-part fill the PE while the current step's cell updates and
attention softmax run on Act/DVE/Pool.

All large DMA transfers are single instructions over host-prelaid [128, N]
images (HWDGE fixed cost ~650ns each makes many small DMAs expensive).

Self-contained: builds the Bass program, shards inputs on the host, runs via
run_bass_kernel_spmd on cores 0-7, reassembles the full [T, B, V] output.
"""
import sys

sys.path.insert(0, "/opt/trn_rl_repo")

import numpy as np
import ml_dtypes

import concourse.bass as bass
import concourse.mybir as mybir
import concourse.tile as tile
from concourse.bass_utils import run_bass_kernel_spmd

F32 = mybir.dt.float32
BF16 = mybir.dt.bfloat16
FP8 = mybir.dt.float8e3
I16 = mybir.dt.int16
AF = mybir.ActivationFunctionType
ALU = mybir.AluOpType

nbf16 = ml_dtypes.bfloat16
nfp8 = ml_dtypes.float8_e3m4
WS = 64.0                   # fp8 weight pre-scale (compensated in activations)
DR = mybir.MatmulPerfMode.DoubleRow

V, E, H = 10000, 512, 1024
T, S, B = 48, 48, 32
PAD, COPY_ID, EPS = 0, 1, 1e-7
NCORES = 8
BL = B // NCORES            # batch per core = 4
G4 = 4 * H                  # 4096 gate width
NVC = 20                    # vocab chunks of 512
VCH = 512
KC_E = E // 128             # 4
KC_H = H // 128             # 8
VKC8 = 80                   # padded vocab chunks (8 super-chunks of 10)

# psum column base per gate type (torch order i,f,g,o), laid out i|f|o|2g so
# the three sigmoids and doubled-g all go through one [0:128] sigmoid
_GCOL = {0: 0, 1: 32, 2: 96, 3: 64}


def _gcol(m):
    return _GCOL[m // 8] + (m % 8) * BL


# ---------------------------------------------------------------- wait split
def _split_wide_waits(nc):
    """walrus CTRL codegen accepts at most 1 sync-wait per instruction; move
    excess waits onto preceding NoOps on the same (in-order) engine."""
    for f in nc.m.functions:
        for bb in f.blocks:
            ins_list = list(bb.instructions)
            out = []
            changed = False
            for ins in ins_list:
                si = getattr(ins, "sync_info", None)
                waits = list(si.on_wait) if si is not None else []
                if len(waits) > 1:
                    excess, keep = waits[:-1], waits[-1:]
                    for w in excess:
                        nop = mybir.InstNoOp(
                            name=f"I-{nc.next_id()}",
                            opcode="NoOp",
                            engine=ins.engine,
                            debug=ins.debug,
                            ins=[],
                            outs=[],
                            sync_info=mybir.SyncInfo(on_wait=[w], on_update=[]),
                        )
                        try:
                            nc.register_instruction(nop, overwrite=True)
                        except Exception:
                            pass
                        out.append(nop)
                        changed = True
                    si.on_wait = keep
                    ins.sync_info = si
                out.append(ins)
            if changed:
                try:
                    bb.instructions = out
                except Exception:
                    bb.instructions.clear()
                    bb.instructions.extend(out)


# ---------------------------------------------------------------- program
def build_program(t_steps=T):
    nc = bass.Bass("TRN2")
    dp = nc.declare_dram_parameter

    NR = t_steps * BL
    mtiles = [(r0, min(128, NR - r0)) for r0 in range(0, NR, 128)]
    NMT = len(mtiles)

    # all weight images are host-prelaid as a flat [128, N] SBUF image
    wfh_d = dp("wfh", [128, KC_H * G4], FP8, isOutput=False)  # (Wf@Wc)[:, :H]^T
    wfs_d = dp("wfs", [16, 128, 2048], FP8, isOutput=False)   # (Wf@Wc)[:, H:]^T
    wh08_d = dp("wh08", [128, KC_H * G4], FP8, isOutput=False)   # W_hh0^T
    wi18_d = dp("wi18", [128, KC_H * G4], FP8, isOutput=False)   # W_ih1^T
    wh18_d = dp("wh18", [128, KC_H * G4], FP8, isOutput=False)   # W_hh1^T
    we08_d = dp("we08", [128, KC_E * G4], FP8, isOutput=False)   # W_ih0[:,:E]^T
    wcb_d = dp("wcb", [128, 2 * KC_H * H], FP8, isOutput=False)  # Wc^T
    wpb_d = dp("wpb", [NVC, 128, KC_H * VCH], FP8, isOutput=False)  # Wp^T by vc
    wkT_d = dp("wkT", [KC_H, 128, KC_H * 128], BF16, isOutput=False)  # Wk^T by mt
    emb_d = dp("emb8", [16, 128, 5 * E], FP8, isOutput=False)   # embed^T chunks
    encIA_d = dp("encIA", [128, H], BF16, isOutput=False)  # enc rows (s*4+b), s<32
    encIB_d = dp("encIB", [64, H], BF16, isOutput=False)   # s in 32..47
    encT_d = dp("encT", [128, KC_H * BL * S], BF16, isOutput=False)
    reftok_d = dp("reftok", [128, NR], F32, isOutput=False)
    vidx_d = dp("vidx", [128, VKC8], F32, isOutput=False)       # p + 128*ch
    iota512_d = dp("iota512", [128, VCH], F32, isOutput=False)
    srcsh_d = dp("srcsh", [128, 2 * NVC], F32, isOutput=False)  # rows (s*4+b)
    pen_d = dp("pen", [BL, S * BL], BF16, isOutput=False)       # penalty incl mask
    h0T_d = dp("h0T", [128, KC_H * BL], FP8, isOutput=False)
    h1T_d = dp("h1T", [128, KC_H * BL], FP8, isOutput=False)
    c0T_d = dp("c0T", [128, KC_H * BL], F32, isOutput=False)
    c1T_d = dp("c1T", [128, KC_H * BL], F32, isOutput=False)
    ident4_d = dp("ident4", [4, 4], BF16, isOutput=False)

    y_d = dp("y", [128, NMT, V], BF16, isOutput=True)  # host reorders + casts

    with tile.TileContext(nc) as tc:
        with tc.tile_pool(name="wres", bufs=1) as wpool, \
             tc.tile_pool(name="dram", bufs=1, space="DRAM") as dpool:

            e_dram = dpool.tile([128, NMT, NVC * VCH], BF16, name="e_dram")

            dma = nc.sync.dma_start

            # ---- outer-resident (survive into phase 2)
            combT = wpool.tile([128, KC_H, NR], FP8, name="combT")
            dsbA = wpool.tile([128, NR], BF16, name="dsbA")
            dsbB = wpool.tile([64, NR], BF16, name="dsbB")
            zbuf = wpool.tile([128, 2 * NVC], F32, name="zbuf")
            cwn = wpool.tile([128, 2], F32, name="cwn")
            cw = wpool.tile([128, 2], F32, name="cw")
            spp = wpool.tile([128, 2], F32, name="spp")
            ceps = wpool.tile([128, 2], F32, name="ceps")
            ident4 = wpool.tile([4, 4], BF16, name="ident4")
            srcsh = wpool.tile([128, 2 * NVC], F32, name="srcsh")
            iota512 = wpool.tile([128, VCH], F32, name="iota512")
            dma(out=ident4[:], in_=ident4_d[:])
            dma(out=srcsh[:], in_=srcsh_d[:])
            dma(out=iota512[:], in_=iota512_d[:])

            # ======== phases 0+1 (scoped pool; weights freed before phase 2)
            with tc.tile_pool(name="ph01", bufs=1) as p1:
                wfh = p1.tile([128, KC_H, G4], FP8, name="wfh")
                QA = p1.tile([128, 32 * 128], BF16, name="QA")
                QB = p1.tile([64, 32 * 128], BF16, name="QB")
                wh0 = p1.tile([128, KC_H, G4], FP8, name="wh0")
                wi1 = p1.tile([128, KC_H, G4], FP8, name="wi1")
                wh1 = p1.tile([128, KC_H, G4], FP8, name="wh1")
                we0 = p1.tile([128, KC_E, G4], FP8, name="we0")
                wcb = p1.tile([128, 2 * KC_H, H], FP8, name="wcb")
                XeT = p1.tile([128, KC_E, NR], FP8, name="XeT")
                attKT = p1.tile([128, KC_H, BL * S], FP8, name="attKT")
                encIA = p1.tile([128, H], BF16, name="encIA")
                encIB = p1.tile([64, H], BF16, name="encIB")
                pen = p1.tile([BL, S * BL], BF16, name="pen")
                h0T = p1.tile([128, KC_H, BL], FP8, name="h0T")
                h1T = p1.tile([128, KC_H, BL], FP8, name="h1T")
                c0T = p1.tile([128, KC_H * BL], F32, name="c0T")
                c1T = p1.tile([128, KC_H * BL], F32, name="c1T")
                combT0 = p1.tile([128, KC_H, BL], FP8, name="combT0")
                sumT = p1.tile([128, KC_H, BL], FP8, name="sumT")

                # small state first, then weights in first-use order
                dma(out=h0T[:], in_=h0T_d[:])
                dma(out=h1T[:], in_=h1T_d[:])
                dma(out=c0T[:], in_=c0T_d[:])
                dma(out=c1T[:], in_=c1T_d[:])
                dma(out=pen[:], in_=pen_d[:])
                dma(out=encIA[:], in_=encIA_d[:])
                dma(out=encIB[:], in_=encIB_d[:])
                nc.vector.memset(combT0[:], 0.0)

                # ---- phase 0a: X_embT = embed^T @ onehot(ref_tokens)
                with tc.tile_pool(name="ph0", bufs=1) as p0:
                    with tc.tile_pool(name="ph0a", bufs=1) as p0a, \
                         tc.tile_pool(name="ps0a", bufs=1, space="PSUM") as ps0a:
                        reftok = p0a.tile([128, NR], F32, name="reftok")
                        vidx = p0a.tile([128, VKC8], F32, name="vidx")
                        dma(out=reftok[:], in_=reftok_d[:])
                        dma(out=vidx[:], in_=vidx_d[:])
                        psX = [ps0a.tile([128, NR], F32, name=f"psX{m}",
                                         tag=f"psX{m}", bufs=1)
                               for m in range(KC_E)]
                        for sc in range(16):
                            emb8 = p0a.tile([128, 5 * E], FP8, name="emb8",
                                            tag="emb8", bufs=2)
                            dma(out=emb8[:], in_=emb_d[sc])
                            for j in range(5):
                                ch = sc * 5 + j
                                oref = p0a.tile([128, NR], BF16, name="oref",
                                                tag="oref", bufs=2)
                                nc.vector.tensor_scalar(
                                    out=oref[:], in0=reftok[:],
                                    scalar1=vidx[:, ch:ch + 1],
                                    scalar2=None, op0=ALU.is_equal)
                                for m in range(KC_E):
                                    nc.tensor.matmul(
                                        psX[m][:],
                                        lhsT=emb8[:, j * E + m * 128:
                                                  j * E + (m + 1) * 128],
                                        rhs=oref[:], start=(ch == 0),
                                        stop=(ch == VKC8 - 1))
                        for m in range(KC_E):
                            nc.vector.tensor_scalar(out=XeT[:, m, :],
                                                    in0=psX[m][:],
                                                    scalar1=1.0 / WS,
                                                    scalar2=None, op0=ALU.mult)

                    # gate weights (one DMA each, first-use order)
                    dma(out=wh0[:], in_=wh08_d[:])
                    dma(out=we0[:], in_=we08_d[:])
                    dma(out=wh1[:], in_=wh18_d[:])
                    dma(out=wi1[:], in_=wi18_d[:])

                    encTs = p0.tile([128, KC_H * BL * S], BF16, name="encTs")
                    dma(out=encTs[:], in_=encT_d[:])

                    # ---- phase 0b: Q^T = WS * enc @ Wfc_s^T  (two jc passes)
                    with tc.tile_pool(name="ps0q", bufs=1, space="PSUM") as ps0q:
                        for ph in range(2):
                            psq = [ps0q.tile([128, VCH], F32, name=f"psq{i}",
                                             tag=f"psq{i}", bufs=1)
                                   for i in range(8)]
                            qtiles = [(0, 128), (128, 64)]
                            for k in range(KC_H):
                                wfsk = p0.tile([128, 2048], FP8, name="wfsk",
                                               tag="wfsk", bufs=2)
                                dma(out=wfsk[:], in_=wfs_d[ph * KC_H + k])
                                for mt2, (r0, mm) in enumerate(qtiles):
                                    for jc in range(4):
                                        nc.tensor.matmul(
                                            psq[mt2 * 4 + jc][:mm, :],
                                            lhsT=encTs[:, k * BL * S + r0:
                                                       k * BL * S + r0 + mm],
                                            rhs=wfsk[:, jc * VCH:(jc + 1) * VCH],
                                            start=(k == 0), stop=(k == KC_H - 1))
                            for mt2, (r0, mm) in enumerate(qtiles):
                                qdst = QA if mt2 == 0 else QB
                                for jc in range(4):
                                    nc.vector.tensor_copy(
                                        out=qdst[:mm, (ph * 4 + jc) * VCH:
                                                 (ph * 4 + jc + 1) * VCH],
                                        in_=psq[mt2 * 4 + jc][:mm, :])

                    dma(out=wfh[:], in_=wfh_d[:])

                    # ---- phase 0c: att_keyT = Wk @ enc^T
                    with tc.tile_pool(name="ps0c", bufs=1, space="PSUM") as ps0c:
                        for mt in range(KC_H):
                            wkmt = p0.tile([128, KC_H * 128], BF16, name="wkmt",
                                           tag="wkmt", bufs=1)
                            dma(out=wkmt[:], in_=wkT_d[mt])
                            psa = ps0c.tile([128, BL * S], F32, name="psa",
                                            tag="psa", bufs=2)
                            for k in range(KC_H):
                                nc.tensor.matmul(
                                    psa[:], lhsT=wkmt[:, k * 128:(k + 1) * 128],
                                    rhs=encTs[:, k * BL * S:(k + 1) * BL * S],
                                    start=(k == 0), stop=(k == KC_H - 1))
                            nc.vector.tensor_copy(out=attKT[:, mt, :], in_=psa[:])

                    dma(out=wcb[:], in_=wcb_d[:])

                # ======== phase 1: software-pipelined recurrence
                SIG, TANH = AF.Sigmoid, AF.Tanh
                with tc.tile_pool(name="gps", bufs=3, space="PSUM") as gps, \
                     tc.tile_pool(name="sps", bufs=1, space="PSUM") as sps:

                    g0t = {}
                    g1t = {}

                    def getg(d, t):
                        if t not in d:
                            d[t] = gps.tile([128, 128], F32, name="g", tag="g",
                                            bufs=3)
                        return d[t]

                    def drmm(g, w, rhs3, kp, m, start, stop):
                        c = _gcol(m)
                        nc.tensor.matmul(
                            g[:, c:c + BL],
                            lhsT=w[:, 2 * kp:2 * kp + 1, m * 128:(m + 1) * 128],
                            rhs=rhs3[:, 0:1, :], start=start, stop=False)
                        nc.tensor.matmul(
                            g[:, c:c + BL],
                            lhsT=w[:, 2 * kp + 1:2 * kp + 2, m * 128:(m + 1) * 128],
                            rhs=rhs3[:, 1:2, :], start=False, stop=stop)

                    def emit_A(t, part):
                        g0 = getg(g0t, t)
                        if part == "xe":
                            w, kk = we0, KC_E // 2
                            rf = lambda kp: XeT[:, 2 * kp:2 * kp + 2,
                                                t * BL:(t + 1) * BL]
                        elif part == "h0":
                            w, kk = wh0, KC_H // 2
                            rf = lambda kp: h0T[:, 2 * kp:2 * kp + 2, :]
                        else:  # "fh": Wfc_h @ h1^{t-1}
                            w, kk = wfh, KC_H // 2
                            rf = lambda kp: h1T[:, 2 * kp:2 * kp + 2, :]
                        first = part == "xe"
                        last = part == "h0" and t == 0
                        for kp in range(kk):
                            rhs = rf(kp)
                            for m in range(32):
                                drmm(g0, w, rhs, kp, m,
                                     first and kp == 0 and m == 0,
                                     last and kp == kk - 1 and m == 31)

                    def emit_Q(t, tdist):  # gates0 += Q^T @ dist^{tdist}
                        g0 = getg(g0t, t)
                        for m in range(32):
                            c = _gcol(m)
                            nc.tensor.matmul(
                                g0[:, c:c + BL], lhsT=QA[:, m * 128:(m + 1) * 128],
                                rhs=dsbA[:, tdist * BL:(tdist + 1) * BL],
                                start=False, stop=False)
                            nc.tensor.matmul(
                                g0[:, c:c + BL], lhsT=QB[:, m * 128:(m + 1) * 128],
                                rhs=dsbB[:, tdist * BL:(tdist + 1) * BL],
                                start=False, stop=(m == 31))

                    def emit_B(t):  # L1 gates, wh1 @ h1^{t-1}
                        g1 = getg(g1t, t)
                        for kp in range(KC_H // 2):
                            rhs = h1T[:, 2 * kp:2 * kp + 2, :]
                            for m in range(32):
                                drmm(g1, wh1, rhs, kp, m, kp == 0 and m == 0,
                                     False)

                    def emit_C(t):  # L1 gates, wi1 @ h0^t
                        g1 = getg(g1t, t)
                        for kp in range(KC_H // 2):
                            rhs = h0T[:, 2 * kp:2 * kp + 2, :]
                            for m in range(32):
                                drmm(g1, wi1, rhs, kp, m, False,
                                     kp == KC_H // 2 - 1 and m == 31)

                    def emit_cell(t, layer):
                        g = (g0t if layer == 0 else g1t).pop(t)
                        cT = c0T if layer == 0 else c1T
                        hT = h0T if layer == 0 else h1T
                        gs = p1.tile([128, 128], F32, name="gs", tag="gs", bufs=2)
                        nc.scalar.activation(out=gs[:], in_=g[:], func=SIG,
                                             scale=1.0 / WS)
                        gg = p1.tile([128, 32], F32, name="gg", tag="gg", bufs=2)
                        nc.vector.tensor_scalar(out=gg[:], in0=gs[:, 96:128],
                                                scalar1=2.0, scalar2=-1.0,
                                                op0=ALU.mult, op1=ALU.add)
                        u = p1.tile([128, 32], F32, name="u", tag="u", bufs=2)
                        nc.vector.tensor_tensor(out=u[:], in0=gs[:, 0:32],
                                                in1=gg[:], op=ALU.mult)
                        nc.vector.tensor_tensor(out=cT[:], in0=gs[:, 32:64],
                                                in1=cT[:], op=ALU.mult)
                        nc.vector.tensor_tensor(out=cT[:], in0=cT[:], in1=u[:],
                                                op=ALU.add)
                        th = p1.tile([128, 32], F32, name="th", tag="th", bufs=2)
                        nc.scalar.activation(out=th[:], in_=cT[:], func=TANH)
                        nc.vector.tensor_tensor(out=hT[:, :, :], in0=gs[:, 64:96],
                                                in1=th[:], op=ALU.mult)

                    def emit_att(t):
                        pss = sps.tile([BL, BL * S], F32, name="pss", tag="pss",
                                       bufs=1)
                        for k in range(KC_H):
                            nc.tensor.matmul(
                                pss[:], lhsT=h1T[:, k:k + 1, :],
                                rhs=attKT[:, k:k + 1, :],
                                start=(k == 0), stop=False)
                        nc.tensor.matmul(pss[:], lhsT=ident4[:], rhs=pen[:],
                                         start=False, stop=True)
                        ssum = p1.tile([BL, 1], F32, name="ssum", tag="ssum",
                                       bufs=2)
                        dstc = p1.tile([BL, S * BL], F32, name="dstc", tag="dstc",
                                       bufs=2)
                        nc.scalar.activation(out=dstc[:], in_=pss[:], func=AF.Exp,
                                             accum_out=ssum[:])
                        rs = p1.tile([BL, 1], F32, name="rs", tag="rs", bufs=2)
                        nc.vector.reciprocal(out=rs[:], in_=ssum[:])
                        dstb = p1.tile([BL, S * BL], BF16, name="dstb", tag="dstb",
                                       bufs=2)
                        nc.vector.tensor_scalar(out=dstb[:], in0=dstc[:],
                                                scalar1=rs[:], scalar2=None,
                                                op0=ALU.mult)
                        return dstb

                    def emit_sumcomb(t, dstb):
                        psDA = sps.tile([128, BL], BF16, name="psDA", tag="psT",
                                        bufs=1)
                        nc.tensor.transpose(psDA[:], dstb[:, 0:128], ident4[:])
                        nc.vector.tensor_copy(out=dsbA[:, t * BL:(t + 1) * BL],
                                              in_=psDA[:])
                        psDB = sps.tile([64, BL], BF16, name="psDB", tag="psTB",
                                        bufs=1)
                        nc.tensor.transpose(psDB[:], dstb[:, 128:192], ident4[:])
                        nc.scalar.copy(out=dsbB[:, t * BL:(t + 1) * BL],
                                       in_=psDB[:])
                        pssu = sps.tile([128, KC_H * BL], F32, name="pssu",
                                        tag="pssu", bufs=1)
                        for j in range(KC_H):
                            nc.tensor.matmul(
                                pssu[:, j * BL:(j + 1) * BL],
                                lhsT=encIA[:, j * 128:(j + 1) * 128],
                                rhs=dsbA[:, t * BL:(t + 1) * BL],
                                start=(j == 0), stop=False)
                            nc.tensor.matmul(
                                pssu[:, j * BL:(j + 1) * BL],
                                lhsT=encIB[:, j * 128:(j + 1) * 128],
                                rhs=dsbB[:, t * BL:(t + 1) * BL],
                                start=False, stop=(j == KC_H - 1))
                        nc.vector.tensor_copy(out=sumT[:, :, :], in_=pssu[:])
                        psc = sps.tile([128, KC_H * BL], F32, name="psc",
                                       tag="psc", bufs=1)
                        for k in range(KC_H):
                            rhs = h1T[:, k:k + 1, :]
                            for mcc in range(KC_H):
                                nc.tensor.matmul(
                                    psc[:, mcc * BL:(mcc + 1) * BL],
                                    lhsT=wcb[:, k:k + 1,
                                             mcc * 128:(mcc + 1) * 128],
                                    rhs=rhs, start=(k == 0 and mcc == 0),
                                    stop=False)
                        for k in range(KC_H):
                            rhs = sumT[:, k:k + 1, :]
                            for mcc in range(KC_H):
                                nc.tensor.matmul(
                                    psc[:, mcc * BL:(mcc + 1) * BL],
                                    lhsT=wcb[:, KC_H + k:KC_H + k + 1,
                                             mcc * 128:(mcc + 1) * 128],
                                    rhs=rhs, start=False,
                                    stop=(k == KC_H - 1 and mcc == KC_H - 1))
                        nc.vector.tensor_scalar(
                            out=combT[:, :, t * BL:(t + 1) * BL], in0=psc[:],
                            scalar1=1.0 / WS, scalar2=None, op0=ALU.mult)

                    for t in range(t_steps):
                        if t == 0:
                            emit_A(0, "xe")
                            emit_A(0, "h0")
                            emit_B(0)
                        emit_cell(t, 0)
                        emit_C(t)
                        if t + 1 < t_steps:
                            emit_A(t + 1, "xe")
                            emit_A(t + 1, "h0")
                        emit_cell(t, 1)
                        if t + 1 < t_steps:
                            emit_A(t + 1, "fh")
                        dstb = emit_att(t)
                        if t + 1 < t_steps:
                            emit_B(t + 1)
                        emit_sumcomb(t, dstb)
                        if t + 1 < t_steps:
                            emit_Q(t + 1, t)

            # ======== phase 2 (own pools; vc outer so Wp streams once)
            with tc.tile_pool(name="ph2", bufs=1) as p2, \
                 tc.tile_pool(name="ps2", bufs=2, space="PSUM") as ps2:
                # one-hot tiles, SBUF-resident fp8 (generated here on DVE)
                ohA = p2.tile([128, NVC * VCH], BF16, name="ohA")
                ohB = p2.tile([64, NVC * VCH], BF16, name="ohB")
                for tl, nrow, oh in ((0, 128, ohA), (1, 64, ohB)):
                    for ch in range(NVC):
                        nc.vector.tensor_scalar(
                            out=oh[:nrow, ch * VCH:(ch + 1) * VCH],
                            in0=iota512[:nrow, :],
                            scalar1=srcsh[:nrow, tl * NVC + ch:tl * NVC + ch + 1],
                            scalar2=None, op0=ALU.is_equal)

                for vc in range(NVC):
                    vlim = min(VCH, V - vc * VCH)
                    wpc = p2.tile([128, KC_H, VCH], FP8, name="wpc", tag="wpc",
                                  bufs=2)
                    dma(out=wpc[:], in_=wpb_d[vc])
                    esb = p2.tile([128, NMT, VCH], BF16, name="esb", tag="esb",
                                  bufs=2)
                    for mt, (r0, mm) in enumerate(mtiles):
                        if mm < 128:
                            nc.vector.memset(esb[:, mt, :], 0.0)
                        psp = ps2.tile([128, VCH], F32, name="psp", tag="psg",
                                       bufs=2)
                        for k in range(KC_H):
                            nc.tensor.matmul(
                                psp[:mm, :],
                                lhsT=combT[:, k:k + 1, r0:r0 + mm],
                                rhs=wpc[:, k:k + 1, :],
                                start=(k == 0), stop=(k == KC_H - 1))
                        nc.scalar.activation(out=esb[:mm, mt, :vlim],
                                             in_=psp[:mm, :vlim],
                                             func=AF.Exp, scale=1.0 / WS,
                                             accum_out=zbuf[:mm, mt * NVC + vc:
                                                            mt * NVC + vc + 1])
                        if vc == 0:
                            nc.scalar.activation(out=cwn[:mm, mt:mt + 1],
                                                 in_=psp[:mm, COPY_ID:COPY_ID + 1],
                                                 func=AF.Exp, scale=1.0 / WS)
                    dma(out=e_dram[:, :, vc * VCH:vc * VCH + vlim],
                        in_=esb[:, :, :vlim])
                for mt, (r0, mm) in enumerate(mtiles):
                    zt = p2.tile([128, 1], F32, name="zt", tag="zt", bufs=2)
                    nc.vector.tensor_reduce(out=zt[:mm, :],
                                            in_=zbuf[:mm, mt * NVC:(mt + 1) * NVC],
                                            op=ALU.add, axis=mybir.AxisListType.X)
                    iz = p2.tile([128, 1], F32, name="iz", tag="zt", bufs=2)
                    nc.vector.reciprocal(out=iz[:mm, :], in_=zt[:mm, :])
                    nc.vector.tensor_tensor(out=cw[:mm, mt:mt + 1],
                                            in0=cwn[:mm, mt:mt + 1], in1=iz[:mm, :],
                                            op=ALU.mult)
                    omc = p2.tile([128, 1], F32, name="omc", tag="zt", bufs=2)
                    nc.vector.tensor_scalar(out=omc[:mm, :], in0=cw[:mm, mt:mt + 1],
                                            scalar1=-1.0, scalar2=1.0,
                                            op0=ALU.mult, op1=ALU.add)
                    nc.vector.tensor_tensor(out=spp[:mm, mt:mt + 1], in0=omc[:mm, :],
                                            in1=iz[:mm, :], op=ALU.mult)
                    nc.vector.tensor_scalar(out=ceps[:mm, mt:mt + 1],
                                            in0=cw[:mm, mt:mt + 1],
                                            scalar1=EPS, scalar2=None, op0=ALU.mult)
                for vc in range(NVC):
                    vlim = min(VCH, V - vc * VCH)
                    e2 = p2.tile([128, NMT, VCH], BF16, name="e2", tag="esb",
                                 bufs=2)
                    dma(out=e2[:, :, :vlim],
                        in_=e_dram[:, :, vc * VCH:vc * VCH + vlim])
                    outc = p2.tile([128, NMT, VCH], BF16, name="outc", tag="outc",
                                   bufs=2)
                    for mt, (r0, mm) in enumerate(mtiles):
                        if mm < 128:
                            nc.vector.memset(outc[:, mt, :], 0.0)
                        pscp = ps2.tile([128, VCH], F32, name="pscp", tag="psg",
                                        bufs=2)
                        nc.tensor.matmul(pscp[:mm, :vlim],
                                         lhsT=dsbA[:, r0:r0 + mm],
                                         rhs=ohA[:, vc * VCH:vc * VCH + vlim],
                                         start=True, stop=False)
                        nc.tensor.matmul(pscp[:mm, :vlim],
                                         lhsT=dsbB[:, r0:r0 + mm],
                                         rhs=ohB[:, vc * VCH:vc * VCH + vlim],
                                         start=False, stop=True)
                        nc.vector.tensor_scalar(out=pscp[:mm, :vlim],
                                                in0=pscp[:mm, :vlim],
                                                scalar1=cw[:mm, mt:mt + 1],
                                                scalar2=ceps[:mm, mt:mt + 1],
                                                op0=ALU.mult, op1=ALU.add)
                        ppf = p2.tile([128, VCH], F32, name="ppf", tag="ppf",
                                      bufs=2)
                        nc.vector.tensor_scalar(out=ppf[:mm, :vlim],
                                                in0=e2[:mm, mt, :vlim],
                                                scalar1=spp[:mm, mt:mt + 1],
                                                scalar2=None, op0=ALU.mult)
                        nc.vector.tensor_tensor(out=ppf[:mm, :vlim],
                                                in0=ppf[:mm, :vlim],
                                                in1=pscp[:mm, :vlim], op=ALU.add)
                        nc.scalar.activation(out=outc[:mm, mt, :vlim],
                                             in_=ppf[:mm, :vlim], func=AF.Ln)
                    dma(out=y_d[:, :, vc * VCH:vc * VCH + vlim],
                        in_=outc[:, :, :vlim])

    _split_wide_waits(nc)
    return nc


# ---------------------------------------------------------------- host prep
def _flatkT(w, dtype):
    """[K, N] -> flat SBUF image [128, (K//128)*N]"""
    K = w.shape[0]
    c = np.ascontiguousarray(w.reshape(K // 128, 128, -1))
    return np.ascontiguousarray(c.transpose(1, 0, 2).reshape(128, -1)).astype(dtype)


def _featmaj(x):
    """[BL, H] -> [128, KC_H*BL] feature-major (chunk-blocked transpose)"""
    xT = x.T  # [H, BL]
    return np.ascontiguousarray(
        xT.reshape(KC_H, 128, BL).transpose(1, 0, 2).reshape(128, KC_H * BL))


def prep_core_inputs(inputs, c, t_steps=T):
    ii = {k: np.asarray(v) for k, v in inputs.items()}
    Bc = list(range(c * BL, (c + 1) * BL))
    W_ih0 = ii["W_ih0"].astype(np.float32).copy()
    W_hh0 = ii["W_hh0"].astype(np.float32).copy()
    W_ih1 = ii["W_ih1"].astype(np.float32).copy()
    W_hh1 = ii["W_hh1"].astype(np.float32).copy()
    # fold tanh(g) = 2*sigmoid(2g)-1: double the g-gate weight rows
    for W in (W_ih0, W_hh0, W_ih1, W_hh1):
        W[2 * H:3 * H, :] *= 2.0
    Wc = ii["Wc"].astype(np.float32)
    Wp = ii["Wp"].astype(np.float32)
    Wk = ii["Wk"].astype(np.float32)
    enc = ii["enc_features"].astype(np.float32)
    embed = ii["embed"].astype(np.float32)
    rt, st = ii["ref_tokens"], ii["src_tokens"]

    Wfc = (W_ih0[:, E:] @ Wc).astype(np.float32)   # feed folded through Wc
    d = {}
    d["wfh"] = _flatkT(WS * Wfc[:, :H].T, nfp8)
    wfsT = np.ascontiguousarray((WS * Wfc[:, H:]).T)  # [H, G4]
    d["wfs"] = np.ascontiguousarray(
        wfsT.reshape(KC_H, 128, 2, 2048).transpose(2, 0, 1, 3)
        .reshape(16, 128, 2048)).astype(nfp8)
    d["wh08"] = _flatkT(WS * W_hh0.T, nfp8)
    d["wi18"] = _flatkT(WS * W_ih1.T, nfp8)
    d["wh18"] = _flatkT(WS * W_hh1.T, nfp8)
    d["we08"] = _flatkT(WS * W_ih0[:, :E].T, nfp8)
    d["wcb"] = _flatkT(WS * Wc.T, nfp8)
    # Wp^T padded, regrouped [vc][p][k*512+c]
    wpT = np.zeros((H, NVC * VCH), np.float32)
    wpT[:, :V] = Wp.T
    d["wpb"] = np.ascontiguousarray(
        (WS * wpT).reshape(KC_H, 128, NVC, VCH).transpose(2, 1, 0, 3)
        .reshape(NVC, 128, KC_H * VCH)).astype(nfp8)
    # Wk^T regrouped [mt][p][k*128+cc]
    d["wkT"] = np.ascontiguousarray(
        Wk.T.reshape(KC_H, 128, KC_H, 128).transpose(2, 1, 0, 3)
        .reshape(KC_H, 128, KC_H * 128)).astype(nbf16)
    # embed padded to 80 chunks, super-chunks of 10: [sc][p][j*E+c]
    embp = np.zeros((VKC8 * 128, E), np.float32)
    embp[:V] = embed
    d["emb8"] = np.ascontiguousarray(
        (WS * embp).reshape(16, 5, 128, E).transpose(0, 2, 1, 3)
        .reshape(16, 128, 5 * E)).astype(nfp8)
    # enc interleaved rows (s*4+b): tile A s<32, tile B s>=32
    encI = enc[:, Bc, :].reshape(S * BL, H)  # row s*BL+b
    d["encIA"] = np.ascontiguousarray(encI[0:128]).astype(nbf16)
    d["encIB"] = np.ascontiguousarray(encI[128:192]).astype(nbf16)
    # encT flat [p][k*192+(s,b)]
    encT = enc[:, Bc, :].transpose(2, 0, 1).reshape(H, S * BL)
    d["encT"] = _flatkT(encT, nbf16)
    # reftok replicated: col (t*BL + b)
    rtc = rt[:t_steps][:, Bc].astype(np.float32).reshape(t_steps * BL)
    d["reftok"] = np.tile(rtc[None, :], (128, 1)).astype(np.float32)
    d["vidx"] = (np.arange(128)[:, None]
                 + 128 * np.arange(VKC8)[None, :]).astype(np.float32)
    d["iota512"] = np.tile(np.arange(VCH, dtype=np.float32)[None, :], (128, 1))
    # srcsh [128, 2*NVC]: rows (s*4+b); tile 0: s<32, tile 1: s>=32
    stI = st[:, Bc].reshape(S * BL).astype(np.float32)  # row s*4+b
    srcsh = np.zeros((128, 2 * NVC), np.float32)
    for ch in range(NVC):
        srcsh[:, ch] = stI[0:128] - VCH * ch
        srcsh[0:64, NVC + ch] = stI[128:192] - VCH * ch
    d["srcsh"] = srcsh
    # pen_full [4, (s*4+b)]: row bp, col (s,b): -99999*mask if b==bp else -99999
    penf = np.full((BL, S * BL), -99999.0, np.float32)
    for bp in range(BL):
        penf[bp, bp::BL] = -99999.0 * (st[:, Bc[bp]] == PAD).astype(np.float32)
    d["pen"] = penf.astype(nbf16)
    h0 = ii["h0"].astype(np.float32)
    c0 = ii["c0"].astype(np.float32)
    d["h0T"] = _featmaj(h0[0][Bc]).astype(nfp8)
    d["h1T"] = _featmaj(h0[1][Bc]).astype(nfp8)
    d["c0T"] = _featmaj(c0[0][Bc]).astype(np.float32)
    d["c1T"] = _featmaj(c0[1][Bc]).astype(np.float32)
    d["ident4"] = np.eye(4, dtype=nbf16)
    # biases must be zero for this kernel (spec fill=zeros)
    for bn in ("bk", "bc", "bp", "b_ih0", "b_hh0", "b_ih1", "b_hh1"):
        assert np.abs(np.asarray(ii[bn])).max() == 0.0, f"nonzero bias {bn}"
    return d


def unpack_y(arr, t_steps=T):
    """[128, NMT, V] bf16 -> [t_steps, BL, V] f32"""
    NR = t_steps * BL
    nmt = arr.shape[1]
    flat = np.ascontiguousarray(
        np.asarray(arr).transpose(1, 0, 2)).reshape(nmt * 128, V)
    return flat[:NR].reshape(t_steps, BL, V).astype(np.float32)


def kernel(**inputs):
    t_steps = np.asarray(inputs["ref_tokens"]).shape[0]
    nc = build_program(t_steps)
    in_maps = [prep_core_inputs(inputs, c, t_steps) for c in range(NCORES)]
    res = run_bass_kernel_spmd(nc, in_maps, list(range(NCORES)))
    out = np.zeros((t_steps, B, V), np.float32)
    for c in range(NCORES):
        out[:, c * BL:(c + 1) * BL, :] = unpack_y(
            np.asarray(res.results[c]["y"]), t_steps)
    return out


if __name__ == "__main__":
    pass


# revision 23
# speedup vs baseline: 1.0570x; 1.0345x over previous
"""Trainium2 Bass kernel for nn_Decoder (LSTM decoder + attention + copy mechanism).

Strategy: pure batch-parallel across the 8 NeuronCores — each core runs the
full T=48-step recurrence and the vocab projection for its 4 batch elements,
with zero cross-core communication (this runtime exposes none).

The recurrence runs in feature-major layout: gates/hidden/cell live as
[feature-chunk(128-part), batch] tiles, the gate weights are the STATIONARY
matmul operand (lhsT, fp8 e3m4 resident in SBUF) and the batch-4 activations
stream as the 4-column moving operand, so a gate matmul costs 4 PE rows
instead of 512. tanh(g) is folded into one full-width sigmoid by pre-doubling
the g-gate weight rows on the host (tanh(x) = 2*sigmoid(2x)-1). The per-step
emission is software-pipelined: the next step's embedding/h0 gate matmuls and
layer-1 h1-part fill the PE while the current step's cell updates and
attention softmax run on Act/DVE/Pool.

All large DMA transfers are single instructions over host-prelaid [128, N]
images (HWDGE fixed cost ~650ns each makes many small DMAs expensive).

Self-contained: builds the Bass program, shards inputs on the host, runs via
run_bass_kernel_spmd on cores 0-7, reassembles the full [T, B, V] output.
"""
import sys

sys.path.insert(0, "/opt/trn_rl_repo")

import numpy as np
import ml_dtypes

import concourse.bass as bass
import concourse.mybir as mybir
import concourse.tile as tile
from concourse.bass_utils import run_bass_kernel_spmd

F32 = mybir.dt.float32
BF16 = mybir.dt.bfloat16
FP8 = mybir.dt.float8e3
I16 = mybir.dt.int16
AF = mybir.ActivationFunctionType
ALU = mybir.AluOpType

nbf16 = ml_dtypes.bfloat16
nfp8 = ml_dtypes.float8_e3m4
WS = 64.0                   # fp8 weight pre-scale (compensated in activations)
DR = mybir.MatmulPerfMode.DoubleRow

V, E, H = 10000, 512, 1024
T, S, B = 48, 48, 32
PAD, COPY_ID, EPS = 0, 1, 1e-7
NCORES = 8
BL = B // NCORES            # batch per core = 4
G4 = 4 * H                  # 4096 gate width
NVC = 20                    # vocab chunks of 512
VCH = 512
KC_E = E // 128             # 4
KC_H = H // 128             # 8
VKC8 = 80                   # padded vocab chunks (8 super-chunks of 10)

# psum column base per gate type (torch order i,f,g,o), laid out i|f|o|2g so
# the three sigmoids and doubled-g all go through one [0:128] sigmoid
_GCOL = {0: 0, 1: 32, 2: 96, 3: 64}


def _gcol(m):
    return _GCOL[m // 8] + (m % 8) * BL


# ---------------------------------------------------------------- wait split
def _split_wide_waits(nc):
    """walrus CTRL codegen accepts at most 1 sync-wait per instruction; move
    excess waits onto preceding NoOps on the same (in-order) engine."""
    for f in nc.m.functions:
        for bb in f.blocks:
            ins_list = list(bb.instructions)
            out = []
            changed = False
            for ins in ins_list:
                si = getattr(ins, "sync_info", None)
                waits = list(si.on_wait) if si is not None else []
                if len(waits) > 1:
                    excess, keep = waits[:-1], waits[-1:]
                    for w in excess:
                        nop = mybir.InstNoOp(
                            name=f"I-{nc.next_id()}",
                            opcode="NoOp",
                            engine=ins.engine,
                            debug=ins.debug,
                            ins=[],
                            outs=[],
                            sync_info=mybir.SyncInfo(on_wait=[w], on_update=[]),
                        )
                        try:
                            nc.register_instruction(nop, overwrite=True)
                        except Exception:
                            pass
                        out.append(nop)
                        changed = True
                    si.on_wait = keep
                    ins.sync_info = si
                out.append(ins)
            if changed:
                try:
                    bb.instructions = out
                except Exception:
                    bb.instructions.clear()
                    bb.instructions.extend(out)


# ---------------------------------------------------------------- program
def build_program(t_steps=T):
    nc = bass.Bass("TRN2")
    dp = nc.declare_dram_parameter

    NR = t_steps * BL
    mtiles = [(r0, min(128, NR - r0)) for r0 in range(0, NR, 128)]
    NMT = len(mtiles)

    # all weight images are host-prelaid as a flat [128, N] SBUF image
    wfh_d = dp("wfh", [128, KC_H * G4], FP8, isOutput=False)  # (Wf@Wc)[:, :H]^T
    wfs_d = dp("wfs", [16, 128, 2048], FP8, isOutput=False)   # (Wf@Wc)[:, H:]^T
    wh08_d = dp("wh08", [128, KC_H * G4], FP8, isOutput=False)   # W_hh0^T
    wi18_d = dp("wi18", [128, KC_H * G4], FP8, isOutput=False)   # W_ih1^T
    wh18_d = dp("wh18", [128, KC_H * G4], FP8, isOutput=False)   # W_hh1^T
    we08_d = dp("we08", [128, KC_E * G4], FP8, isOutput=False)   # W_ih0[:,:E]^T
    wcb_d = dp("wcb", [128, 2 * KC_H * H], FP8, isOutput=False)  # Wc^T
    wpb_d = dp("wpb", [NVC, 128, KC_H * VCH], FP8, isOutput=False)  # Wp^T by vc
    wkT_d = dp("wkT", [KC_H, 128, KC_H * 128], BF16, isOutput=False)  # Wk^T by mt
    emb_d = dp("emb8", [16, 128, 5 * E], FP8, isOutput=False)   # embed^T chunks
    encIA_d = dp("encIA", [128, H], BF16, isOutput=False)  # enc rows (s*4+b), s<32
    encIB_d = dp("encIB", [64, H], BF16, isOutput=False)   # s in 32..47
    encT_d = dp("encT", [128, KC_H * BL * S], BF16, isOutput=False)
    reftok_d = dp("reftok", [128, NR], F32, isOutput=False)
    vidx_d = dp("vidx", [128, VKC8], F32, isOutput=False)       # p + 128*ch
    iota512_d = dp("iota512", [128, VCH], F32, isOutput=False)
    srcsh_d = dp("srcsh", [128, 2 * NVC], F32, isOutput=False)  # rows (s*4+b)
    pen_d = dp("pen", [BL, S * BL], BF16, isOutput=False)       # penalty incl mask
    h0T_d = dp("h0T", [128, KC_H * BL], FP8, isOutput=False)
    h1T_d = dp("h1T", [128, KC_H * BL], FP8, isOutput=False)
    c0T_d = dp("c0T", [128, KC_H * BL], F32, isOutput=False)
    c1T_d = dp("c1T", [128, KC_H * BL], F32, isOutput=False)
    ident4_d = dp("ident4", [4, 4], BF16, isOutput=False)

    y_d = dp("y", [128, NMT, V], BF16, isOutput=True)  # host reorders + casts

    with tile.TileContext(nc) as tc:
        with tc.tile_pool(name="wres", bufs=1) as wpool, \
             tc.tile_pool(name="dram", bufs=1, space="DRAM") as dpool:

            e_dram = dpool.tile([128, NMT, NVC * VCH], BF16, name="e_dram")

            dma = nc.sync.dma_start

            # ---- outer-resident (survive into phase 2)
            combT = wpool.tile([128, KC_H, NR], FP8, name="combT")
            dsbA = wpool.tile([128, NR], BF16, name="dsbA")
            dsbB = wpool.tile([64, NR], BF16, name="dsbB")
            zbuf = wpool.tile([128, 2 * NVC], F32, name="zbuf")
            cwn = wpool.tile([128, 2], F32, name="cwn")
            cw = wpool.tile([128, 2], F32, name="cw")
            spp = wpool.tile([128, 2], F32, name="spp")
            ceps = wpool.tile([128, 2], F32, name="ceps")
            ident4 = wpool.tile([4, 4], BF16, name="ident4")
            srcsh = wpool.tile([128, 2 * NVC], F32, name="srcsh")
            iota512 = wpool.tile([128, VCH], F32, name="iota512")
            dma(out=ident4[:], in_=ident4_d[:])
            dma(out=srcsh[:], in_=srcsh_d[:])
            dma(out=iota512[:], in_=iota512_d[:])

            # ======== phases 0+1 (scoped pool; weights freed before phase 2)
            with tc.tile_pool(name="ph01", bufs=1) as p1:
                wfh = p1.tile([128, KC_H, G4], FP8, name="wfh")
                QA = p1.tile([128, 32 * 128], BF16, name="QA")
                QB = p1.tile([64, 32 * 128], BF16, name="QB")
                wh0 = p1.tile([128, KC_H, G4], FP8, name="wh0")
                wi1 = p1.tile([128, KC_H, G4], FP8, name="wi1")
                wh1 = p1.tile([128, KC_H, G4], FP8, name="wh1")
                we0 = p1.tile([128, KC_E, G4], FP8, name="we0")
                wcb = p1.tile([128, 2 * KC_H, H], FP8, name="wcb")
                XeT = p1.tile([128, KC_E, NR], FP8, name="XeT")
                attKT = p1.tile([128, KC_H, BL * S], FP8, name="attKT")
                encIA = p1.tile([128, H], BF16, name="encIA")
                encIB = p1.tile([64, H], BF16, name="encIB")
                pen = p1.tile([BL, S * BL], BF16, name="pen")
                h0T = p1.tile([128, KC_H, BL], FP8, name="h0T")
                h1T = p1.tile([128, KC_H, BL], FP8, name="h1T")
                c0T = p1.tile([128, KC_H * BL], F32, name="c0T")
                c1T = p1.tile([128, KC_H * BL], F32, name="c1T")
                combT0 = p1.tile([128, KC_H, BL], FP8, name="combT0")
                sumT = p1.tile([128, KC_H, BL], FP8, name="sumT")

                # small state first, then weights in first-use order
                dma(out=h0T[:], in_=h0T_d[:])
                dma(out=h1T[:], in_=h1T_d[:])
                dma(out=c0T[:], in_=c0T_d[:])
                dma(out=c1T[:], in_=c1T_d[:])
                dma(out=pen[:], in_=pen_d[:])
                dma(out=encIA[:], in_=encIA_d[:])
                dma(out=encIB[:], in_=encIB_d[:])
                nc.vector.memset(combT0[:], 0.0)

                # ---- phase 0a: X_embT = embed^T @ onehot(ref_tokens)
                with tc.tile_pool(name="ph0", bufs=1) as p0:
                    with tc.tile_pool(name="ph0a", bufs=1) as p0a, \
                         tc.tile_pool(name="ps0a", bufs=1, space="PSUM") as ps0a:
                        reftok = p0a.tile([128, NR], F32, name="reftok")
                        vidx = p0a.tile([128, VKC8], F32, name="vidx")
                        dma(out=reftok[:], in_=reftok_d[:])
                        dma(out=vidx[:], in_=vidx_d[:])
                        psX = [ps0a.tile([128, NR], F32, name=f"psX{m}",
                                         tag=f"psX{m}", bufs=1)
                               for m in range(KC_E)]
                        for sc in range(16):
                            emb8 = p0a.tile([128, 5 * E], FP8, name="emb8",
                                            tag="emb8", bufs=2)
                            dma(out=emb8[:], in_=emb_d[sc])
                            for j in range(5):
                                ch = sc * 5 + j
                                oref = p0a.tile([128, NR], BF16, name="oref",
                                                tag="oref", bufs=2)
                                nc.vector.tensor_scalar(
                                    out=oref[:], in0=reftok[:],
                                    scalar1=vidx[:, ch:ch + 1],
                                    scalar2=None, op0=ALU.is_equal)
                                for m in range(KC_E):
                                    nc.tensor.matmul(
                                        psX[m][:],
                                        lhsT=emb8[:, j * E + m * 128:
                                                  j * E + (m + 1) * 128],
                                        rhs=oref[:], start=(ch == 0),
                                        stop=(ch == VKC8 - 1))
                        for m in range(KC_E):
                            nc.vector.tensor_scalar(out=XeT[:, m, :],
                                                    in0=psX[m][:],
                                                    scalar1=1.0 / WS,
                                                    scalar2=None, op0=ALU.mult)

                    # gate weights (one DMA each, first-use order)
                    dma(out=wh0[:], in_=wh08_d[:])
                    dma(out=we0[:], in_=we08_d[:])
                    dma(out=wh1[:], in_=wh18_d[:])
                    dma(out=wi1[:], in_=wi18_d[:])

                    encTs = p0.tile([128, KC_H * BL * S], BF16, name="encTs")
                    dma(out=encTs[:], in_=encT_d[:])

                    # ---- phase 0b: Q^T = WS * enc @ Wfc_s^T  (two jc passes)
                    with tc.tile_pool(name="ps0q", bufs=1, space="PSUM") as ps0q:
                        for ph in range(2):
                            psq = [ps0q.tile([128, VCH], F32, name=f"psq{i}",
                                             tag=f"psq{i}", bufs=1)
                                   for i in range(8)]
                            qtiles = [(0, 128), (128, 64)]
                            for k in range(KC_H):
                                wfsk = p0.tile([128, 2048], FP8, name="wfsk",
                                               tag="wfsk", bufs=2)
                                dma(out=wfsk[:], in_=wfs_d[ph * KC_H + k])
                                for mt2, (r0, mm) in enumerate(qtiles):
                                    for jc in range(4):
                                        nc.tensor.matmul(
                                            psq[mt2 * 4 + jc][:mm, :],
                                            lhsT=encTs[:, k * BL * S + r0:
                                                       k * BL * S + r0 + mm],
                                            rhs=wfsk[:, jc * VCH:(jc + 1) * VCH],
                                            start=(k == 0), stop=(k == KC_H - 1))
                            for mt2, (r0, mm) in enumerate(qtiles):
                                qdst = QA if mt2 == 0 else QB
                                for jc in range(4):
                                    nc.vector.tensor_copy(
                                        out=qdst[:mm, (ph * 4 + jc) * VCH:
                                                 (ph * 4 + jc + 1) * VCH],
                                        in_=psq[mt2 * 4 + jc][:mm, :])

                    dma(out=wfh[:], in_=wfh_d[:])

                    # ---- phase 0c: att_keyT = Wk @ enc^T
                    with tc.tile_pool(name="ps0c", bufs=1, space="PSUM") as ps0c:
                        for mt in range(KC_H):
                            wkmt = p0.tile([128, KC_H * 128], BF16, name="wkmt",
                                           tag="wkmt", bufs=1)
                            dma(out=wkmt[:], in_=wkT_d[mt])
                            psa = ps0c.tile([128, BL * S], F32, name="psa",
                                            tag="psa", bufs=2)
                            for k in range(KC_H):
                                nc.tensor.matmul(
                                    psa[:], lhsT=wkmt[:, k * 128:(k + 1) * 128],
                                    rhs=encTs[:, k * BL * S:(k + 1) * BL * S],
                                    start=(k == 0), stop=(k == KC_H - 1))
                            nc.vector.tensor_copy(out=attKT[:, mt, :], in_=psa[:])

                    dma(out=wcb[:], in_=wcb_d[:])

                # ======== phase 1: software-pipelined recurrence
                SIG, TANH = AF.Sigmoid, AF.Tanh
                with tc.tile_pool(name="gps", bufs=3, space="PSUM") as gps, \
                     tc.tile_pool(name="sps", bufs=1, space="PSUM") as sps:

                    g0t = {}
                    g1t = {}

                    def getg(d, t):
                        if t not in d:
                            d[t] = gps.tile([128, 128], F32, name="g", tag="g",
                                            bufs=3)
                        return d[t]

                    def drmm(g, w, rhs3, kp, m, start, stop):
                        c = _gcol(m)
                        nc.tensor.matmul(
                            g[:, c:c + BL],
                            lhsT=w[:, 2 * kp:2 * kp + 1, m * 128:(m + 1) * 128],
                            rhs=rhs3[:, 0:1, :], start=start, stop=False)
                        nc.tensor.matmul(
                            g[:, c:c + BL],
                            lhsT=w[:, 2 * kp + 1:2 * kp + 2, m * 128:(m + 1) * 128],
                            rhs=rhs3[:, 1:2, :], start=False, stop=stop)

                    def emit_A(t, part):
                        g0 = getg(g0t, t)
                        if part == "xe":
                            w, kk = we0, KC_E // 2
                            rf = lambda kp: XeT[:, 2 * kp:2 * kp + 2,
                                                t * BL:(t + 1) * BL]
                        elif part == "h0":
                            w, kk = wh0, KC_H // 2
                            rf = lambda kp: h0T[:, 2 * kp:2 * kp + 2, :]
                        else:  # "fh": Wfc_h @ h1^{t-1}
                            w, kk = wfh, KC_H // 2
                            rf = lambda kp: h1T[:, 2 * kp:2 * kp + 2, :]
                        first = part == "xe"
                        last = part == "h0" and t == 0
                        for kp in range(kk):
                            rhs = rf(kp)
                            for m in range(32):
                                drmm(g0, w, rhs, kp, m,
                                     first and kp == 0 and m == 0,
                                     last and kp == kk - 1 and m == 31)

                    def emit_Q(t, tdist):  # gates0 += Q^T @ dist^{tdist}
                        g0 = getg(g0t, t)
                        for m in range(32):
                            c = _gcol(m)
                            nc.tensor.matmul(
                                g0[:, c:c + BL], lhsT=QA[:, m * 128:(m + 1) * 128],
                                rhs=dsbA[:, tdist * BL:(tdist + 1) * BL],
                                start=False, stop=False)
                            nc.tensor.matmul(
                                g0[:, c:c + BL], lhsT=QB[:, m * 128:(m + 1) * 128],
                                rhs=dsbB[:, tdist * BL:(tdist + 1) * BL],
                                start=False, stop=(m == 31))

                    def emit_B(t):  # L1 gates, wh1 @ h1^{t-1}
                        g1 = getg(g1t, t)
                        for kp in range(KC_H // 2):
                            rhs = h1T[:, 2 * kp:2 * kp + 2, :]
                            for m in range(32):
                                drmm(g1, wh1, rhs, kp, m, kp == 0 and m == 0,
                                     False)

                    def emit_C(t):  # L1 gates, wi1 @ h0^t
                        g1 = getg(g1t, t)
                        for kp in range(KC_H // 2):
                            rhs = h0T[:, 2 * kp:2 * kp + 2, :]
                            for m in range(32):
                                drmm(g1, wi1, rhs, kp, m, False,
                                     kp == KC_H // 2 - 1 and m == 31)

                    def emit_cell(t, layer):
                        g = (g0t if layer == 0 else g1t).pop(t)
                        cT = c0T if layer == 0 else c1T
                        hT = h0T if layer == 0 else h1T
                        gs = p1.tile([128, 128], F32, name="gs", tag="gs", bufs=2)
                        nc.scalar.activation(out=gs[:], in_=g[:], func=SIG,
                                             scale=1.0 / WS)
                        gg = p1.tile([128, 32], F32, name="gg", tag="gg", bufs=2)
                        nc.vector.tensor_scalar(out=gg[:], in0=gs[:, 96:128],
                                                scalar1=2.0, scalar2=-1.0,
                                                op0=ALU.mult, op1=ALU.add)
                        u = p1.tile([128, 32], F32, name="u", tag="u", bufs=2)
                        nc.vector.tensor_tensor(out=u[:], in0=gs[:, 0:32],
                                                in1=gg[:], op=ALU.mult)
                        nc.vector.tensor_tensor(out=cT[:], in0=gs[:, 32:64],
                                                in1=cT[:], op=ALU.mult)
                        nc.vector.tensor_tensor(out=cT[:], in0=cT[:], in1=u[:],
                                                op=ALU.add)
                        th = p1.tile([128, 32], F32, name="th", tag="th", bufs=2)
                        nc.scalar.activation(out=th[:], in_=cT[:], func=TANH)
                        nc.vector.tensor_tensor(out=hT[:, :, :], in0=gs[:, 64:96],
                                                in1=th[:], op=ALU.mult)

                    def emit_att(t):
                        pss = sps.tile([BL, BL * S], F32, name="pss", tag="pss",
                                       bufs=1)
                        for k in range(KC_H):
                            nc.tensor.matmul(
                                pss[:], lhsT=h1T[:, k:k + 1, :],
                                rhs=attKT[:, k:k + 1, :],
                                start=(k == 0), stop=False)
                        nc.tensor.matmul(pss[:], lhsT=ident4[:], rhs=pen[:],
                                         start=False, stop=True)
                        ssum = p1.tile([BL, 1], F32, name="ssum", tag="ssum",
                                       bufs=2)
                        dstc = p1.tile([BL, S * BL], F32, name="dstc", tag="dstc",
                                       bufs=2)
                        nc.scalar.activation(out=dstc[:], in_=pss[:], func=AF.Exp,
                                             accum_out=ssum[:])
                        rs = p1.tile([BL, 1], F32, name="rs", tag="rs", bufs=2)
                        nc.vector.reciprocal(out=rs[:], in_=ssum[:])
                        dstb = p1.tile([BL, S * BL], BF16, name="dstb", tag="dstb",
                                       bufs=2)
                        nc.vector.tensor_scalar(out=dstb[:], in0=dstc[:],
                                                scalar1=rs[:], scalar2=None,
                                                op0=ALU.mult)
                        return dstb

                    def emit_sumcomb(t, dstb):
                        psDA = sps.tile([128, BL], BF16, name="psDA", tag="psT",
                                        bufs=1)
                        nc.tensor.transpose(psDA[:], dstb[:, 0:128], ident4[:])
                        nc.vector.tensor_copy(out=dsbA[:, t * BL:(t + 1) * BL],
                                              in_=psDA[:])
                        psDB = sps.tile([64, BL], BF16, name="psDB", tag="psTB",
                                        bufs=1)
                        nc.tensor.transpose(psDB[:], dstb[:, 128:192], ident4[:])
                        nc.scalar.copy(out=dsbB[:, t * BL:(t + 1) * BL],
                                       in_=psDB[:])
                        pssu = sps.tile([128, KC_H * BL], F32, name="pssu",
                                        tag="pssu", bufs=1)
                        for j in range(KC_H):
                            nc.tensor.matmul(
                                pssu[:, j * BL:(j + 1) * BL],
                                lhsT=encIA[:, j * 128:(j + 1) * 128],
                                rhs=dsbA[:, t * BL:(t + 1) * BL],
                                start=(j == 0), stop=False)
                            nc.tensor.matmul(
                                pssu[:, j * BL:(j + 1) * BL],
                                lhsT=encIB[:, j * 128:(j + 1) * 128],
                                rhs=dsbB[:, t * BL:(t + 1) * BL],
                                start=False, stop=(j == KC_H - 1))
                        nc.vector.tensor_copy(out=sumT[:, :, :], in_=pssu[:])
                        psc = sps.tile([128, KC_H * BL], F32, name="psc",
                                       tag="psc", bufs=1)
                        for k in range(KC_H):
                            rhs = h1T[:, k:k + 1, :]
                            for mcc in range(KC_H):
                                nc.tensor.matmul(
                                    psc[:, mcc * BL:(mcc + 1) * BL],
                                    lhsT=wcb[:, k:k + 1,
                                             mcc * 128:(mcc + 1) * 128],
                                    rhs=rhs, start=(k == 0 and mcc == 0),
                                    stop=False)
                        for k in range(KC_H):
                            rhs = sumT[:, k:k + 1, :]
                            for mcc in range(KC_H):
                                nc.tensor.matmul(
                                    psc[:, mcc * BL:(mcc + 1) * BL],
                                    lhsT=wcb[:, KC_H + k:KC_H + k + 1,
                                             mcc * 128:(mcc + 1) * 128],
                                    rhs=rhs, start=False,
                                    stop=(k == KC_H - 1 and mcc == KC_H - 1))
                        nc.vector.tensor_scalar(
                            out=combT[:, :, t * BL:(t + 1) * BL], in0=psc[:],
                            scalar1=1.0 / WS, scalar2=None, op0=ALU.mult)

                    for t in range(t_steps):
                        if t == 0:
                            emit_A(0, "xe")
                            emit_A(0, "h0")
                            emit_B(0)
                        emit_cell(t, 0)
                        emit_C(t)
                        if t + 1 < t_steps:
                            emit_A(t + 1, "xe")
                            emit_A(t + 1, "h0")
                        emit_cell(t, 1)
                        dstb = emit_att(t)
                        if t + 1 < t_steps:
                            emit_A(t + 1, "fh")
                            emit_B(t + 1)
                        emit_sumcomb(t, dstb)
                        if t + 1 < t_steps:
                            emit_Q(t + 1, t)

            # ======== phase 2 (own pools; vc outer so Wp streams once)
            with tc.tile_pool(name="ph2", bufs=1) as p2, \
                 tc.tile_pool(name="ps2", bufs=2, space="PSUM") as ps2:
                # one-hot tiles, SBUF-resident fp8 (generated here on DVE)
                ohA = p2.tile([128, NVC * VCH], BF16, name="ohA")
                ohB = p2.tile([64, NVC * VCH], BF16, name="ohB")
                for tl, nrow, oh in ((0, 128, ohA), (1, 64, ohB)):
                    for ch in range(NVC):
                        nc.vector.tensor_scalar(
                            out=oh[:nrow, ch * VCH:(ch + 1) * VCH],
                            in0=iota512[:nrow, :],
                            scalar1=srcsh[:nrow, tl * NVC + ch:tl * NVC + ch + 1],
                            scalar2=None, op0=ALU.is_equal)

                for vc in range(NVC):
                    vlim = min(VCH, V - vc * VCH)
                    wpc = p2.tile([128, KC_H, VCH], FP8, name="wpc", tag="wpc",
                                  bufs=2)
                    dma(out=wpc[:], in_=wpb_d[vc])
                    esb = p2.tile([128, NMT, VCH], BF16, name="esb", tag="esb",
                                  bufs=2)
                    for mt, (r0, mm) in enumerate(mtiles):
                        if mm < 128:
                            nc.vector.memset(esb[:, mt, :], 0.0)
                        psp = ps2.tile([128, VCH], F32, name="psp", tag="psg",
                                       bufs=2)
                        for k in range(KC_H):
                            nc.tensor.matmul(
                                psp[:mm, :],
                                lhsT=combT[:, k:k + 1, r0:r0 + mm],
                                rhs=wpc[:, k:k + 1, :],
                                start=(k == 0), stop=(k == KC_H - 1))
                        nc.scalar.activation(out=esb[:mm, mt, :vlim],
                                             in_=psp[:mm, :vlim],
                                             func=AF.Exp, scale=1.0 / WS,
                                             accum_out=zbuf[:mm, mt * NVC + vc:
                                                            mt * NVC + vc + 1])
                        if vc == 0:
                            nc.scalar.activation(out=cwn[:mm, mt:mt + 1],
                                                 in_=psp[:mm, COPY_ID:COPY_ID + 1],
                                                 func=AF.Exp, scale=1.0 / WS)
                    dma(out=e_dram[:, :, vc * VCH:vc * VCH + vlim],
                        in_=esb[:, :, :vlim])
                for mt, (r0, mm) in enumerate(mtiles):
                    zt = p2.tile([128, 1], F32, name="zt", tag="zt", bufs=2)
                    nc.vector.tensor_reduce(out=zt[:mm, :],
                                            in_=zbuf[:mm, mt * NVC:(mt + 1) * NVC],
                                            op=ALU.add, axis=mybir.AxisListType.X)
                    iz = p2.tile([128, 1], F32, name="iz", tag="zt", bufs=2)
                    nc.vector.reciprocal(out=iz[:mm, :], in_=zt[:mm, :])
                    nc.vector.tensor_tensor(out=cw[:mm, mt:mt + 1],
                                            in0=cwn[:mm, mt:mt + 1], in1=iz[:mm, :],
                                            op=ALU.mult)
                    omc = p2.tile([128, 1], F32, name="omc", tag="zt", bufs=2)
                    nc.vector.tensor_scalar(out=omc[:mm, :], in0=cw[:mm, mt:mt + 1],
                                            scalar1=-1.0, scalar2=1.0,
                                            op0=ALU.mult, op1=ALU.add)
                    nc.vector.tensor_tensor(out=spp[:mm, mt:mt + 1], in0=omc[:mm, :],
                                            in1=iz[:mm, :], op=ALU.mult)
                    nc.vector.tensor_scalar(out=ceps[:mm, mt:mt + 1],
                                            in0=cw[:mm, mt:mt + 1],
                                            scalar1=EPS, scalar2=None, op0=ALU.mult)
                for vc in range(NVC):
                    vlim = min(VCH, V - vc * VCH)
                    e2 = p2.tile([128, NMT, VCH], BF16, name="e2", tag="esb",
                                 bufs=2)
                    dma(out=e2[:, :, :vlim],
                        in_=e_dram[:, :, vc * VCH:vc * VCH + vlim])
                    outc = p2.tile([128, NMT, VCH], BF16, name="outc", tag="outc",
                                   bufs=2)
                    for mt, (r0, mm) in enumerate(mtiles):
                        if mm < 128:
                            nc.vector.memset(outc[:, mt, :], 0.0)
                        pscp = ps2.tile([128, VCH], F32, name="pscp", tag="psg",
                                        bufs=2)
                        nc.tensor.matmul(pscp[:mm, :vlim],
                                         lhsT=dsbA[:, r0:r0 + mm],
                                         rhs=ohA[:, vc * VCH:vc * VCH + vlim],
                                         start=True, stop=False)
                        nc.tensor.matmul(pscp[:mm, :vlim],
                                         lhsT=dsbB[:, r0:r0 + mm],
                                         rhs=ohB[:, vc * VCH:vc * VCH + vlim],
                                         start=False, stop=True)
                        nc.vector.tensor_scalar(out=pscp[:mm, :vlim],
                                                in0=pscp[:mm, :vlim],
                                                scalar1=cw[:mm, mt:mt + 1],
                                                scalar2=ceps[:mm, mt:mt + 1],
                                                op0=ALU.mult, op1=ALU.add)
                        ppf = p2.tile([128, VCH], F32, name="ppf", tag="ppf",
                                      bufs=2)
                        nc.vector.tensor_scalar(out=ppf[:mm, :vlim],
                                                in0=e2[:mm, mt, :vlim],
                                                scalar1=spp[:mm, mt:mt + 1],
                                                scalar2=None, op0=ALU.mult)
                        nc.vector.tensor_tensor(out=ppf[:mm, :vlim],
                                                in0=ppf[:mm, :vlim],
                                                in1=pscp[:mm, :vlim], op=ALU.add)
                        nc.scalar.activation(out=outc[:mm, mt, :vlim],
                                             in_=ppf[:mm, :vlim], func=AF.Ln)
                    dma(out=y_d[:, :, vc * VCH:vc * VCH + vlim],
                        in_=outc[:, :, :vlim])

    _split_wide_waits(nc)
    return nc


# ---------------------------------------------------------------- host prep
def _flatkT(w, dtype):
    """[K, N] -> flat SBUF image [128, (K//128)*N]"""
    K = w.shape[0]
    c = np.ascontiguousarray(w.reshape(K // 128, 128, -1))
    return np.ascontiguousarray(c.transpose(1, 0, 2).reshape(128, -1)).astype(dtype)


def _featmaj(x):
    """[BL, H] -> [128, KC_H*BL] feature-major (chunk-blocked transpose)"""
    xT = x.T  # [H, BL]
    return np.ascontiguousarray(
        xT.reshape(KC_H, 128, BL).transpose(1, 0, 2).reshape(128, KC_H * BL))


def prep_core_inputs(inputs, c, t_steps=T):
    ii = {k: np.asarray(v) for k, v in inputs.items()}
    Bc = list(range(c * BL, (c + 1) * BL))
    W_ih0 = ii["W_ih0"].astype(np.float32).copy()
    W_hh0 = ii["W_hh0"].astype(np.float32).copy()
    W_ih1 = ii["W_ih1"].astype(np.float32).copy()
    W_hh1 = ii["W_hh1"].astype(np.float32).copy()
    # fold tanh(g) = 2*sigmoid(2g)-1: double the g-gate weight rows
    for W in (W_ih0, W_hh0, W_ih1, W_hh1):
        W[2 * H:3 * H, :] *= 2.0
    Wc = ii["Wc"].astype(np.float32)
    Wp = ii["Wp"].astype(np.float32)
    Wk = ii["Wk"].astype(np.float32)
    enc = ii["enc_features"].astype(np.float32)
    embed = ii["embed"].astype(np.float32)
    rt, st = ii["ref_tokens"], ii["src_tokens"]

    Wfc = (W_ih0[:, E:] @ Wc).astype(np.float32)   # feed folded through Wc
    d = {}
    d["wfh"] = _flatkT(WS * Wfc[:, :H].T, nfp8)
    wfsT = np.ascontiguousarray((WS * Wfc[:, H:]).T)  # [H, G4]
    d["wfs"] = np.ascontiguousarray(
        wfsT.reshape(KC_H, 128, 2, 2048).transpose(2, 0, 1, 3)
        .reshape(16, 128, 2048)).astype(nfp8)
    d["wh08"] = _flatkT(WS * W_hh0.T, nfp8)
    d["wi18"] = _flatkT(WS * W_ih1.T, nfp8)
    d["wh18"] = _flatkT(WS * W_hh1.T, nfp8)
    d["we08"] = _flatkT(WS * W_ih0[:, :E].T, nfp8)
    d["wcb"] = _flatkT(WS * Wc.T, nfp8)
    # Wp^T padded, regrouped [vc][p][k*512+c]
    wpT = np.zeros((H, NVC * VCH), np.float32)
    wpT[:, :V] = Wp.T
    d["wpb"] = np.ascontiguousarray(
        (WS * wpT).reshape(KC_H, 128, NVC, VCH).transpose(2, 1, 0, 3)
        .reshape(NVC, 128, KC_H * VCH)).astype(nfp8)
    # Wk^T regrouped [mt][p][k*128+cc]
    d["wkT"] = np.ascontiguousarray(
        Wk.T.reshape(KC_H, 128, KC_H, 128).transpose(2, 1, 0, 3)
        .reshape(KC_H, 128, KC_H * 128)).astype(nbf16)
    # embed padded to 80 chunks, super-chunks of 10: [sc][p][j*E+c]
    embp = np.zeros((VKC8 * 128, E), np.float32)
    embp[:V] = embed
    d["emb8"] = np.ascontiguousarray(
        (WS * embp).reshape(16, 5, 128, E).transpose(0, 2, 1, 3)
        .reshape(16, 128, 5 * E)).astype(nfp8)
    # enc interleaved rows (s*4+b): tile A s<32, tile B s>=32
    encI = enc[:, Bc, :].reshape(S * BL, H)  # row s*BL+b
    d["encIA"] = np.ascontiguousarray(encI[0:128]).astype(nbf16)
    d["encIB"] = np.ascontiguousarray(encI[128:192]).astype(nbf16)
    # encT flat [p][k*192+(s,b)]
    encT = enc[:, Bc, :].transpose(2, 0, 1).reshape(H, S * BL)
    d["encT"] = _flatkT(encT, nbf16)
    # reftok replicated: col (t*BL + b)
    rtc = rt[:t_steps][:, Bc].astype(np.float32).reshape(t_steps * BL)
    d["reftok"] = np.tile(rtc[None, :], (128, 1)).astype(np.float32)
    d["vidx"] = (np.arange(128)[:, None]
                 + 128 * np.arange(VKC8)[None, :]).astype(np.float32)
    d["iota512"] = np.tile(np.arange(VCH, dtype=np.float32)[None, :], (128, 1))
    # srcsh [128, 2*NVC]: rows (s*4+b); tile 0: s<32, tile 1: s>=32
    stI = st[:, Bc].reshape(S * BL).astype(np.float32)  # row s*4+b
    srcsh = np.zeros((128, 2 * NVC), np.float32)
    for ch in range(NVC):
        srcsh[:, ch] = stI[0:128] - VCH * ch
        srcsh[0:64, NVC + ch] = stI[128:192] - VCH * ch
    d["srcsh"] = srcsh
    # pen_full [4, (s*4+b)]: row bp, col (s,b): -99999*mask if b==bp else -99999
    penf = np.full((BL, S * BL), -99999.0, np.float32)
    for bp in range(BL):
        penf[bp, bp::BL] = -99999.0 * (st[:, Bc[bp]] == PAD).astype(np.float32)
    d["pen"] = penf.astype(nbf16)
    h0 = ii["h0"].astype(np.float32)
    c0 = ii["c0"].astype(np.float32)
    d["h0T"] = _featmaj(h0[0][Bc]).astype(nfp8)
    d["h1T"] = _featmaj(h0[1][Bc]).astype(nfp8)
    d["c0T"] = _featmaj(c0[0][Bc]).astype(np.float32)
    d["c1T"] = _featmaj(c0[1][Bc]).astype(np.float32)
    d["ident4"] = np.eye(4, dtype=nbf16)
    # biases must be zero for this kernel (spec fill=zeros)
    for bn in ("bk", "bc", "bp", "b_ih0", "b_hh0", "b_ih1", "b_hh1"):
        assert np.abs(np.asarray(ii[bn])).max() == 0.0, f"nonzero bias {bn}"
    return d


def unpack_y(arr, t_steps=T):
    """[128, NMT, V] bf16 -> [t_steps, BL, V] f32"""
    NR = t_steps * BL
    nmt = arr.shape[1]
    flat = np.ascontiguousarray(
        np.asarray(arr).transpose(1, 0, 2)).reshape(nmt * 128, V)
    return flat[:NR].reshape(t_steps, BL, V).astype(np.float32)


def kernel(**inputs):
    t_steps = np.asarray(inputs["ref_tokens"]).shape[0]
    nc = build_program(t_steps)
    in_maps = [prep_core_inputs(inputs, c, t_steps) for c in range(NCORES)]
    res = run_bass_kernel_spmd(nc, in_maps, list(range(NCORES)))
    out = np.zeros((t_steps, B, V), np.float32)
    for c in range(NCORES):
        out[:, c * BL:(c + 1) * BL, :] = unpack_y(
            np.asarray(res.results[c]["y"]), t_steps)
    return out


if __name__ == "__main__":
    pass
